# revision 32
# baseline (speedup 1.0000x reference)
"""GAT layer (project + edge-softmax attention + aggregate + head-mean + LayerNorm + PReLU)
on 8 Trainium2 NeuronCores.

Sharding: nodes/edges partitioned by destination across the 8 cores; edges of
each core are grouped into 128-destination blocks and 128-edge tiles, tiles
into 32-tile streamed chunks. Per chunk the attention logits are batched:
alpha = x_e@V + smt.T@a_dst accumulates in one PSUM bank via two small PE
matmuls per tile, then one chunk-level leaky-relu (DVE STT) + exp (scalar
engine, single activation table -> no table reloads) produce e; a strided
scalar-engine copy drops the softmax-denominator columns of every tile into
the per-chunk aggregation operand. The whole alpha pipeline for chunk c+1 is
emitted at the midpoint of chunk c so none of it sits on a chunk boundary.
Per tile the projection h_e = x[src_e] @ W runs on PE into bank-aligned PSUM
pair tiles [128, 2, 512]; one DVE broadcast multiply per PAIR forms the
weighted messages h_e * e for both tiles (amortizing the ~120-cycle PSUM read
overhead), and a one-hot mask matmul accumulates messages + denominators per
destination block. The epilogue is interleaved: per block-group (reciprocal,
head-mean, LayerNorm reduces) on DVE as soon as a group's blocks finish, with
a two-stage tail (normalize + PReLU + output DMA for blocks 0-44 hidden under
the last tiles; remainder after the loop). LayerNorm scale-invariance absorbs
the 1/HEADS head-mean factor; trivial affine constants (bias=0, gamma=1,
beta=0) and the PReLU weight are baked at compile time (cache-keyed), with
PReLU as max(y, w*y) for 0 < w < 1.

The host side (input sharding) expands source features per edge slot
(x.T[:, src[slot]], fp16) and ships the one-hot destination masks as fp8
(exact 0/1 index data) so the device consumes purely sequential streams --
per-edge DMA gathers are descriptor-rate-bound (~14 ns/descriptor measured)
on TRN2 and cannot reach the memory roofline, and on-device mask construction
is DVE-bound.
"""
import sys

sys.path.insert(0, "/opt/trn_rl_repo")

import numpy as np
from contextlib import ExitStack

import concourse.bass as bass
import concourse.tile as tile
from concourse import bacc, mybir
from concourse.bass_utils import run_bass_kernel_spmd

# ---- problem constants (hardcoded per harness contract) ----
N = 50000
IN_DIM = 128
OUT_DIM = 64
HEADS = 4
HC = HEADS * OUT_DIM          # 256
NEG_SLOPE = 0.2
EPS = 1e-5

NCORES = 8
ND = N // NCORES              # 6250 dst nodes per core
P = 128
NB = (ND + P - 1) // P        # 49 blocks (last has 106 dsts)
NDP = NB * P                  # 6272 padded local nodes
CH = 40                       # tiles per streamed chunk

F16 = mybir.dt.float16
F32 = mybir.dt.float32
F8 = mybir.dt.float8e4

_CACHE = {}


def _build(S, T_b, pw, triv):
    """Compile the SPMD program. S = padded edge slots per core (mult of 128),
    T_b = tuple of per-block tile counts (len NB, sum*128 == S), pw = PReLU
    weight baked as an immediate (0 < pw < 1 required by the max-form),
    triv = bias==0 & gamma==1 & beta==0 (skips the corresponding epilogue
    ops)."""
    n_tiles = S // P
    RW = HC + HEADS           # 260: rhs/psum width (256 msg + 4 denom cols)

    nc = bacc.Bacc("TRN2", target_bir_lowering=False, debug=False)

    xeT = nc.dram_tensor("xeT", [P, S], F16, kind="ExternalInput")
    smaskd = nc.dram_tensor("smask", [P, S], F8, kind="ExternalInput")
    smtd = nc.dram_tensor("smt", [P, S], F8, kind="ExternalInput")
    xTl = nc.dram_tensor("xTl", [P, NDP], F16, kind="ExternalInput")
    W16d = nc.dram_tensor("W16", [P, HC], F16, kind="ExternalInput")
    V16d = nc.dram_tensor("V16", [P, HEADS], F16, kind="ExternalInput")
    U16d = nc.dram_tensor("U16", [P, HEADS], F16, kind="ExternalInput")
    # packed per-channel constants replicated across partitions:
    # [bias(64) | gamma(64) | beta(64) | prelu_w(1)]
    crep = nc.dram_tensor("crep", [P, 3 * OUT_DIM + 1], F32, kind="ExternalInput")
    out = nc.dram_tensor("out", [NDP, OUT_DIM], F32, kind="ExternalOutput")

    with tile.TileContext(nc) as tc, ExitStack() as ctx:
        const_p = ctx.enter_context(tc.tile_pool(name="const", bufs=1))
        xet_p = ctx.enter_context(tc.tile_pool(name="xet", bufs=3))
        rhs_p = ctx.enter_context(tc.tile_pool(name="rhs", bufs=2))
        ach_p = ctx.enter_context(tc.tile_pool(name="ach", bufs=2))
        epi_p = ctx.enter_context(tc.tile_pool(name="epi", bufs=1))
        ph_p = ctx.enter_context(tc.tile_pool(name="ph", bufs=2, space="PSUM"))
        pm_p = ctx.enter_context(tc.tile_pool(name="pm", bufs=3, space="PSUM"))
        pa_p = ctx.enter_context(tc.tile_pool(name="pa", bufs=1, space="PSUM"))

        # ---- constants ----
        w_s = const_p.tile([P, HC], F16)
        nc.sync.dma_start(w_s[:], W16d[:])
        v_s = const_p.tile([P, HEADS], F16)
        nc.sync.dma_start(v_s[:], V16d[:])
        u_s = const_p.tile([P, HEADS], F16)
        nc.sync.dma_start(u_s[:], U16d[:])
        cr_s = const_p.tile([P, 3 * OUT_DIM + 1], F32)
        nc.sync.dma_start(cr_s[:], crep[:])
        w_prelu = cr_s[:, 3 * OUT_DIM:3 * OUT_DIM + 1]

        # big accumulators for the batched epilogue
        acc_all = const_p.tile([P, NB, RW], F32)      # raw psum copies

        # ---- main loop ----
        # tile -> (block, is_first_in_block, is_last_in_block)
        tinfo = []
        for b, nt in enumerate(T_b):
            for ti in range(nt):
                tinfo.append((b, ti == 0, ti == nt - 1))

        # ramped chunk sizes: small first chunks so the edge pipeline starts
        # before the full stream depth is resident (start is DMA-contended)
        bounds = [0, 8, 24]
        while bounds[-1] + CH < n_tiles:
            bounds.append(bounds[-1] + CH)
        bounds.append(n_tiles)
        if bounds[-1] == bounds[-2]:
            bounds.pop()
        nchunks_r = len(bounds) - 1

        def load_dma(c):
            lo = bounds[c] * P
            hi = bounds[c + 1] * P
            w = hi - lo
            xet_ch = xet_p.tile([P, CH * P], F16, tag="xet")
            nc.sync.dma_start(xet_ch[:, :w], xeT[:, lo:hi])
            sm_ch = xet_p.tile([P, CH * P], F8, tag="smask")
            nc.sync.dma_start(sm_ch[:, :w], smaskd[:, lo:hi])
            smt_ch = xet_p.tile([P, CH * P], F8, tag="smt")
            nc.sync.dma_start(smt_ch[:, :w], smtd[:, lo:hi])
            return xet_ch, sm_ch, smt_ch

        def process_chunk(c):
            ctiles = bounds[c + 1] - bounds[c]
            xet_ch, sm_ch, smt_ch = dma_cache.pop(c) if c in dma_cache \
                else load_dma(c)
            if c == 0:
                # same ordering trick: chunk-1's rings fire after chunk-0's
                ot = ach_p.tile([P, 1], F8, tag="ord")
                nc.sync.dma_start(ot[:], smt_ch[:, 0:1])
            # prefetch the next chunk's streams (bufs=3 keeps DMA ahead)
            if c + 1 < nchunks_r and c + 1 not in dma_cache:
                dma_cache[c + 1] = load_dma(c + 1)

            # alpha for the whole chunk: one PSUM bank, element-wise groups
            pa = pa_p.tile([P, CH * HEADS], F32, space="PSUM")
            for ti in range(ctiles):
                t = bounds[c] + ti
                b = tinfo[t][0]
                asl = slice(ti * HEADS, (ti + 1) * HEADS)
                nc.tensor.matmul(pa[:, asl], lhsT=xet_ch[:, ti * P:(ti + 1) * P],
                                 rhs=v_s[:],
                                 start=(ti == 0), stop=False,
                                 skip_group_check=True)
                nc.tensor.matmul(
                    pa[:, asl], lhsT=smt_ch[:, ti * P:(ti + 1) * P],
                    rhs=adst_s[:, b * HEADS:(b + 1) * HEADS],
                    start=False, stop=(ti == ctiles - 1),
                    skip_group_check=True)
            # chunk-batched leaky + exp (one ACT instr, no table switches in
            # steady state); STT can read only one PSUM operand, so copy out
            a_ch = ach_p.tile([P, CH * HEADS], F32, tag="a_ch")
            nc.scalar.copy(a_ch[:, :ctiles * HEADS], pa[:, :ctiles * HEADS])
            lk_ch = ach_p.tile([P, CH * HEADS], F32, tag="lk_ch")
            nc.vector.scalar_tensor_tensor(
                out=lk_ch[:, :ctiles * HEADS], in0=a_ch[:, :ctiles * HEADS],
                scalar=NEG_SLOPE, in1=a_ch[:, :ctiles * HEADS],
                op0=mybir.AluOpType.mult, op1=mybir.AluOpType.max)
            e_ch = ach_p.tile([P, CH * HEADS], F32, tag="e_ch")
            nc.scalar.activation(e_ch[:, :ctiles * HEADS],
                                 lk_ch[:, :ctiles * HEADS],
                                 mybir.ActivationFunctionType.Exp)
            # denom columns for every tile of the chunk: one strided ACT copy
            # into the big per-chunk aggregation operand (table-free)
            rhs_ch = rhs_p.tile([P, CH, RW], F16, tag="rhs")
            nc.scalar.copy(
                rhs_ch[:, 0:ctiles, HC:RW],
                e_ch[:, :ctiles * HEADS].rearrange("p (t h) -> p t h", h=HEADS))
            return xet_ch, sm_ch, e_ch, rhs_ch

        # ---- phase 0: a_dst for local nodes (kept in SBUF, fp16) ----
        # (chunk-0 streams start first so their DMA overlaps phase-0 compute)
        adst_s = const_p.tile([P, NB * HEADS], F16)
        with tc.tile_pool(name="p0", bufs=1) as p0_pool:
            xtl_s = p0_pool.tile([P, NDP], F16)
            nc.sync.dma_start(xtl_s[:], xTl[:])
            # ordering shim: this tiny DMA *waits* on xTl's completion, so
            # every chunk dma_start queued behind it on the sync engine fires
            # only after xTl's ring has drained -- the early streams stop
            # contending for HBM bandwidth (phase-0 start was DMA-gated)
            order_t = p0_pool.tile([P, 1], F16)
            nc.sync.dma_start(order_t[:], xtl_s[:, 0:1])
            dma_cache = {}
            dma_cache[0] = load_dma(0)
            for b in range(NB):
                ps = pm_p.tile([P, HEADS], F32, space="PSUM", tag="pm")
                nc.tensor.matmul(
                    ps[:], lhsT=xtl_s[:, b * P:(b + 1) * P], rhs=u_s[:],
                    start=True, stop=True)
                nc.scalar.copy(adst_s[:, b * HEADS:(b + 1) * HEADS], ps[:])

        # per-block-group epilogue bulk (head-mean via softmax reciprocals,
        # square + LN reduces), emitted inside the main loop right after a
        # group's blocks finish so it fills DVE idle windows
        rec = epi_p.tile([P, NB, HEADS], F32)
        macc = epi_p.tile([P, NB, OUT_DIM], F32)
        tmp = epi_p.tile([P, NB, OUT_DIM], F32)
        ssum = epi_p.tile([P, NB], F32)
        ssq = epi_p.tile([P, NB], F32)
        mean = epi_p.tile([P, NB], F32)
        var = epi_p.tile([P, NB], F32)
        m2 = epi_p.tile([P, NB], F32)
        rstd = epi_p.tile([P, NB], F32)
        eps_s = epi_p.tile([P, 1], F32)
        nc.vector.memset(eps_s[:], EPS)

        out_ap_full = bass.AP(out.ap().tensor, 0,
                              [[OUT_DIM, P], [P * OUT_DIM, NB], [1, OUT_DIM]])

        def emit_tail(g0, g1):
            """mean/var -> rstd -> normalize -> PReLU -> store, for blocks
            [g0, g1). One Sqrt per call (one activation-table pair swap)."""
            hb = slice(g0, g1)
            w = g1 - g0
            nc.vector.tensor_scalar(out=mean[:, hb], in0=ssum[:, hb],
                                    scalar1=1.0 / OUT_DIM, scalar2=None,
                                    op0=mybir.AluOpType.mult)
            nc.vector.tensor_scalar(out=var[:, hb], in0=ssq[:, hb],
                                    scalar1=1.0 / OUT_DIM, scalar2=None,
                                    op0=mybir.AluOpType.mult)
            nc.vector.tensor_tensor(out=m2[:, hb], in0=mean[:, hb],
                                    in1=mean[:, hb], op=mybir.AluOpType.mult)
            nc.vector.tensor_tensor(out=var[:, hb], in0=var[:, hb],
                                    in1=m2[:, hb], op=mybir.AluOpType.subtract)
            nc.scalar.activation(rstd[:, hb], var[:, hb],
                                 mybir.ActivationFunctionType.Sqrt,
                                 bias=eps_s[:, 0:1])
            nc.vector.reciprocal(rstd[:, hb], rstd[:, hb])
            mean_b = bass.AP(mean[:].tensor, mean[:].offset + g0,
                             [mean[:].ap[0], [1, w], [0, OUT_DIM]])
            rstd_b = bass.AP(rstd[:].tensor, rstd[:].offset + g0,
                             [rstd[:].ap[0], [1, w], [0, OUT_DIM]])
            nc.vector.tensor_tensor(out=macc[:, hb, :], in0=macc[:, hb, :],
                                    in1=mean_b, op=mybir.AluOpType.subtract)
            nc.vector.tensor_tensor(out=macc[:, hb, :], in0=macc[:, hb, :],
                                    in1=rstd_b, op=mybir.AluOpType.mult)
            if not triv:
                gamma_b = bass.AP(cr_s[:].tensor, cr_s[:].offset + OUT_DIM,
                                  [cr_s[:].ap[0], [0, w], [1, OUT_DIM]])
                beta_b = bass.AP(cr_s[:].tensor, cr_s[:].offset + 2 * OUT_DIM,
                                 [cr_s[:].ap[0], [0, w], [1, OUT_DIM]])
                nc.vector.tensor_tensor(out=macc[:, hb, :], in0=macc[:, hb, :],
                                        in1=gamma_b, op=mybir.AluOpType.mult)
                nc.vector.tensor_tensor(out=macc[:, hb, :], in0=macc[:, hb, :],
                                        in1=beta_b, op=mybir.AluOpType.add)
            # PReLU with 0 < pw < 1: max(y, pw*y)
            nc.vector.scalar_tensor_tensor(
                out=macc[:, hb, :], in0=macc[:, hb, :], scalar=pw,
                in1=macc[:, hb, :], op0=mybir.AluOpType.mult,
                op1=mybir.AluOpType.max)
            out_slice = bass.AP(out_ap_full.tensor, g0 * P * OUT_DIM,
                                [[OUT_DIM, P], [P * OUT_DIM, w], [1, OUT_DIM]])
            nc.sync.dma_start(out_slice, macc[:, hb, :])

        def rec_ap(hd, g0, g1):
            base = rec[:]
            return bass.AP(base.tensor, base.offset + g0 * HEADS + hd,
                           [base.ap[0], [HEADS, g1 - g0], [0, OUT_DIM]])

        def emit_group(g0, g1):
            hb = slice(g0, g1)
            nc.vector.reciprocal(rec[:, hb, :], acc_all[:, hb, HC:RW])
            nc.vector.tensor_tensor(out=macc[:, hb, :],
                                    in0=acc_all[:, hb, 0:OUT_DIM],
                                    in1=rec_ap(0, g0, g1),
                                    op=mybir.AluOpType.mult)
            for hd in range(1, HEADS):
                nc.vector.tensor_tensor(
                    out=tmp[:, hb, :],
                    in0=acc_all[:, hb, hd * OUT_DIM:(hd + 1) * OUT_DIM],
                    in1=rec_ap(hd, g0, g1), op=mybir.AluOpType.mult)
                nc.vector.tensor_add(macc[:, hb, :], macc[:, hb, :],
                                     tmp[:, hb, :])
            if not triv:
                bias_b = bass.AP(cr_s[:].tensor, cr_s[:].offset,
                                 [cr_s[:].ap[0], [0, g1 - g0], [1, OUT_DIM]])
                nc.vector.tensor_tensor(out=macc[:, hb, :], in0=macc[:, hb, :],
                                        in1=bias_b, op=mybir.AluOpType.add)
            nc.vector.tensor_tensor(out=tmp[:, hb, :], in0=macc[:, hb, :],
                                    in1=macc[:, hb, :], op=mybir.AluOpType.mult)
            nc.vector.tensor_reduce(ssum[:, hb], macc[:, hb, :],
                                    mybir.AxisListType.X, mybir.AluOpType.add)
            nc.vector.tensor_reduce(ssq[:, hb], tmp[:, hb, :],
                                    mybir.AxisListType.X, mybir.AluOpType.add)

        GROUPS = (9, 18, 27, 36, 44, 48, NB)
        next_g = 0
        done_g = 0

        processed = {}
        cur_c = -1
        xet_ch = sm_ch = e_ch = rhs_ch = None
        pm = None
        for t0 in range(0, n_tiles, 2):
            npair = min(2, n_tiles - t0)
            if t0 >= bounds[cur_c + 1] if cur_c >= 0 else True:
                c = cur_c + 1 if cur_c >= 0 else 0
                while bounds[c + 1] <= t0:
                    c += 1
                tup = processed.pop(c, None) or process_chunk(c)
                xet_ch, sm_ch, e_ch, rhs_ch = tup
                cur_c = c
            c = cur_c
            toff0 = t0 - bounds[c]
            mid = ((bounds[c] + bounds[c + 1]) // 2) & ~1
            if t0 == mid and c + 1 < nchunks_r and c + 1 not in processed:
                # emit the next chunk's alpha pipeline mid-chunk so none of
                # it sits on the chunk boundary
                processed[c + 1] = process_chunk(c + 1)

            # projections into a bank-aligned PSUM pair tile
            phb = ph_p.tile([P, 2, 512], F32, space="PSUM")
            for j in range(npair):
                sl = slice((toff0 + j) * P, (toff0 + j + 1) * P)
                nc.tensor.matmul(phb[:, j, 0:HC], lhsT=xet_ch[:, sl],
                                 rhs=w_s[:], start=True, stop=True,
                                 skip_group_check=True)

            # rhs[:, :, :256] = h * e (per-head broadcast), one instr per pair
            eb0 = e_ch[:, toff0 * HEADS:(toff0 + npair) * HEADS]
            e_b = bass.AP(eb0.tensor, eb0.offset,
                          [eb0.ap[0], [HEADS, npair], [1, HEADS],
                           [0, OUT_DIM]])
            r0 = rhs_ch[:, toff0:toff0 + npair, 0:HC]
            nc.vector.tensor_tensor(
                out=r0.rearrange("p t (h c) -> p t h c", h=HEADS),
                in0=phb[:, 0:npair, 0:HC].rearrange(
                    "p t (h c) -> p t h c", h=HEADS),
                in1=e_b, op=mybir.AluOpType.mult)

            # segment sums + denominators
            for j in range(npair):
                t = t0 + j
                b, first, last = tinfo[t]
                sl = slice((toff0 + j) * P, (toff0 + j + 1) * P)
                if first:
                    pm = pm_p.tile([P, RW], F32, space="PSUM", tag="pm")
                nc.tensor.matmul(pm[:], lhsT=sm_ch[:, sl],
                                 rhs=rhs_ch[:, toff0 + j, :],
                                 start=first, stop=last)
                if last:
                    nc.scalar.copy(acc_all[:, b, :], pm[:])
                    if b + 1 == GROUPS[next_g]:
                        emit_group(done_g, b + 1)
                        done_g = b + 1
                        next_g += 1
                        if done_g == 44:
                            emit_tail(0, 44)
                        elif done_g == 48:
                            emit_tail(44, 48)

        # ---- epilogue final stage: the last block ----
        emit_tail(48, NB)

    nc.compile()
    return nc


def _prep(x, edge_index, W, att_src, att_dst, bias, gamma, beta, prelu_w):
    """Host-side sharding: self-loops, dst-sort, per-core per-block padding,
    per-edge-slot source-feature expansion (fp16), one-hot mask streams,
    weight folding."""
    src = np.concatenate([edge_index[0], np.arange(N, dtype=edge_index.dtype)])
    dst = np.concatenate([edge_index[1], np.arange(N, dtype=edge_index.dtype)])
    order = np.argsort(dst, kind="stable")
    src = src[order].astype(np.int64)
    dst = dst[order].astype(np.int64)

    # folded attention vectors: a_src = x @ V, a_dst = x @ U
    Wh = W.reshape(IN_DIM, HEADS, OUT_DIM)
    V = np.einsum("khc,hc->kh", Wh, att_src).astype(np.float64)  # [128, H]
    U = np.einsum("khc,hc->kh", Wh, att_dst)                     # [128, H]

    # pad column q: q @ V = -c for every head -> exp weight == 0
    c = 5000.0
    Q, _, _, _ = np.linalg.lstsq(V.T, -c * np.ones(HEADS), rcond=None)
    q16 = Q.astype(np.float16)
    assert np.all(np.isfinite(q16)), "pad vector overflows fp16"
    assert (q16.astype(np.float64) @ V < -500).all(), "pad logits not low enough"

    x16 = x.astype(np.float16)

    # degree-balanced dst placement: assign destinations to (core, block)
    # bins so per-bin edge counts equalize -- the shared tile budget T_b is
    # set by the per-block max across cores, so balance cuts padding tiles.
    import heapq
    deg = np.bincount(dst, minlength=N).astype(np.int64)   # incl. self-loop
    order_d = np.argsort(-deg, kind="stable")
    heap = [(0, k, b) for k in range(NCORES) for b in range(NB)]
    heapq.heapify(heap)
    free = np.full((NCORES, NB), P, dtype=np.int64)
    free[:, NB - 1] = ND - (NB - 1) * P        # last block: 106 real dsts
    core_of_d = np.empty(N, dtype=np.int64)
    blk_of_d = np.empty(N, dtype=np.int64)
    pos_of_d = np.empty(N, dtype=np.int64)
    spill = []
    for d_ in order_d:
        while True:
            s, k, b = heapq.heappop(heap)
            if free[k, b] > 0:
                break
            spill.append((s, k, b))
        core_of_d[d_] = k
        blk_of_d[d_] = b
        pos_of_d[d_] = P - ((ND - (NB - 1) * P) if b == NB - 1 else P) \
            + 0  # placeholder, real pos assigned below
        free[k, b] -= 1
        heapq.heappush(heap, (s + deg[d_], k, b))
    # positions within each bin: stable order of assignment
    pos_of_d[:] = 0
    for k in range(NCORES):
        for b in range(NB):
            sel = np.where((core_of_d == k) & (blk_of_d == b))[0]
            pos_of_d[sel] = np.arange(len(sel))

    core_of = core_of_d[dst]
    counts = np.zeros((NCORES, NB), dtype=np.int64)
    np.add.at(counts, (core_of, blk_of_d[dst]), 1)
    T_b = tuple(int(v) for v in np.ceil(counts.max(axis=0) / P).astype(np.int64))
    S = int(sum(T_b)) * P

    in_maps = []
    W16 = W.astype(np.float16)
    V16 = V.astype(np.float16)
    U16 = U.astype(np.float16)
    crep = np.zeros((P, 3 * OUT_DIM + 1), dtype=np.float32)
    crep[:, 0:OUT_DIM] = bias
    crep[:, OUT_DIM:2 * OUT_DIM] = gamma
    crep[:, 2 * OUT_DIM:3 * OUT_DIM] = beta
    crep[:, 3 * OUT_DIM] = prelu_w[0]

    slot_starts = np.concatenate([[0], np.cumsum(np.array(T_b) * P)])
    import ml_dtypes
    eye8 = np.eye(P, dtype=ml_dtypes.float8_e4m3)
    for k in range(NCORES):
        sel = core_of == k
        src_k, dst_k = src[sel], dst[sel]
        blk_k = blk_of_d[dst_k]

        src_slots = np.zeros(S, dtype=np.int64)
        pad_mask = np.ones(S, dtype=bool)
        dloc = np.full(S, 127, dtype=np.int64)
        o = np.argsort(blk_k, kind="stable")
        src_k, dst_k, blk_k = src_k[o], dst_k[o], blk_k[o]
        bstart = np.searchsorted(blk_k, np.arange(NB + 1))
        for b in range(NB):
            lo, hi = bstart[b], bstart[b + 1]
            n = hi - lo
            s0 = slot_starts[b]
            src_slots[s0:s0 + n] = src_k[lo:hi]
            pad_mask[s0:s0 + n] = False
            dloc[s0:s0 + n] = pos_of_d[dst_k[lo:hi]]

        xe = x16[src_slots]                          # [S, 128]
        xe[pad_mask] = q16
        xeT = np.ascontiguousarray(xe.T)             # [128, S]

        # one-hot masks, both orientations, tile-major along free dim
        oh = eye8[dloc].reshape(S // P, P, P)       # [t, e, d]
        smask = np.ascontiguousarray(
            oh.transpose(1, 0, 2).reshape(P, S))     # [e, (t d)]
        smt = np.ascontiguousarray(
            oh.transpose(2, 0, 1).reshape(P, S))     # [d, (t e)]

        xTl = np.zeros((P, NDP), dtype=np.float16)
        mine = np.where(core_of_d == k)[0]
        rows = blk_of_d[mine] * P + pos_of_d[mine]
        xTl[:, rows] = x16[mine].T

        in_maps.append({
            "xeT": xeT, "smask": smask, "smt": smt, "xTl": xTl,
            "W16": W16, "V16": V16, "U16": U16, "crep": crep,
        })
    outidx = core_of_d * NDP + blk_of_d * P + pos_of_d
    return S, T_b, in_maps, outidx


def kernel(x, edge_index, W, att_src, att_dst, bias, gamma, beta, prelu_w,
           _trace=False):
    x = np.asarray(x, dtype=np.float32)
    edge_index = np.asarray(edge_index)
    S, T_b, in_maps, outidx = _prep(
        x, edge_index, np.asarray(W, np.float32), np.asarray(att_src, np.float32),
        np.asarray(att_dst, np.float32), np.asarray(bias, np.float32),
        np.asarray(gamma, np.float32), np.asarray(beta, np.float32),
        np.asarray(prelu_w, np.float32))

    pw = float(np.asarray(prelu_w).reshape(-1)[0])
    triv = bool(np.all(np.asarray(bias) == 0) and np.all(np.asarray(gamma) == 1)
                and np.all(np.asarray(beta) == 0))
    assert 0.0 < pw < 1.0, "max-form PReLU requires 0 < w < 1"
    key = (S, T_b, pw, triv)
    if key not in _CACHE:
        _CACHE[key] = _build(S, T_b, pw, triv)
    nc = _CACHE[key]

    res = run_bass_kernel_spmd(nc, in_maps, core_ids=list(range(NCORES)),
                               trace=_trace)
    allout = np.concatenate(
        [res.results[k]["out"] for k in range(NCORES)], axis=0)
    out = allout[outidx]
    if _trace:
        kernel.last_exec_time_ns = res.exec_time_ns
    return out


# revision 33
# speedup vs baseline: 1.0021x; 1.0021x over previous
"""GAT layer (project + edge-softmax attention + aggregate + head-mean + LayerNorm + PReLU)
on 8 Trainium2 NeuronCores.

Sharding: nodes/edges partitioned by destination across the 8 cores; edges of
each core are grouped into 128-destination blocks and 128-edge tiles, tiles
into 32-tile streamed chunks. Per chunk the attention logits are batched:
alpha = x_e@V + smt.T@a_dst accumulates in one PSUM bank via two small PE
matmuls per tile, then one chunk-level leaky-relu (DVE STT) + exp (scalar
engine, single activation table -> no table reloads) produce e; a strided
scalar-engine copy drops the softmax-denominator columns of every tile into
the per-chunk aggregation operand. The whole alpha pipeline for chunk c+1 is
emitted at the midpoint of chunk c so none of it sits on a chunk boundary.
Per tile the projection h_e = x[src_e] @ W runs on PE into bank-aligned PSUM
pair tiles [128, 2, 512]; one DVE broadcast multiply per PAIR forms the
weighted messages h_e * e for both tiles (amortizing the ~120-cycle PSUM read
overhead), and a one-hot mask matmul accumulates messages + denominators per
destination block. The epilogue is interleaved: per block-group (reciprocal,
head-mean, LayerNorm reduces) on DVE as soon as a group's blocks finish, with
a two-stage tail (normalize + PReLU + output DMA for blocks 0-44 hidden under
the last tiles; remainder after the loop). LayerNorm scale-invariance absorbs
the 1/HEADS head-mean factor; trivial affine constants (bias=0, gamma=1,
beta=0) and the PReLU weight are baked at compile time (cache-keyed), with
PReLU as max(y, w*y) for 0 < w < 1.

The host side (input sharding) expands source features per edge slot
(x.T[:, src[slot]], fp16) and ships the one-hot destination masks as fp8
(exact 0/1 index data) so the device consumes purely sequential streams --
per-edge DMA gathers are descriptor-rate-bound (~14 ns/descriptor measured)
on TRN2 and cannot reach the memory roofline, and on-device mask construction
is DVE-bound.
"""
import sys

sys.path.insert(0, "/opt/trn_rl_repo")

import numpy as np
from contextlib import ExitStack

import concourse.bass as bass
import concourse.tile as tile
from concourse import bacc, mybir
from concourse.bass_utils import run_bass_kernel_spmd

# ---- problem constants (hardcoded per harness contract) ----
N = 50000
IN_DIM = 128
OUT_DIM = 64
HEADS = 4
HC = HEADS * OUT_DIM          # 256
NEG_SLOPE = 0.2
EPS = 1e-5

NCORES = 8
ND = N // NCORES              # 6250 dst nodes per core
P = 128
NB = (ND + P - 1) // P        # 49 blocks (last has 106 dsts)
NDP = NB * P                  # 6272 padded local nodes
CH = 32                       # tiles per streamed chunk

F16 = mybir.dt.float16
F32 = mybir.dt.float32
F8 = mybir.dt.float8e4

_CACHE = {}


def _build(S, T_b, pw, triv):
    """Compile the SPMD program. S = padded edge slots per core (mult of 128),
    T_b = tuple of per-block tile counts (len NB, sum*128 == S), pw = PReLU
    weight baked as an immediate (0 < pw < 1 required by the max-form),
    triv = bias==0 & gamma==1 & beta==0 (skips the corresponding epilogue
    ops)."""
    n_tiles = S // P
    RW = HC + HEADS           # 260: rhs/psum width (256 msg + 4 denom cols)

    nc = bacc.Bacc("TRN2", target_bir_lowering=False, debug=False)

    xeT = nc.dram_tensor("xeT", [P, S], F16, kind="ExternalInput")
    smaskd = nc.dram_tensor("smask", [P, S], F8, kind="ExternalInput")
    smtd = nc.dram_tensor("smt", [P, S], F8, kind="ExternalInput")
    xTl = nc.dram_tensor("xTl", [P, NDP], F16, kind="ExternalInput")
    W16d = nc.dram_tensor("W16", [P, HC], F16, kind="ExternalInput")
    V16d = nc.dram_tensor("V16", [P, HEADS], F16, kind="ExternalInput")
    U16d = nc.dram_tensor("U16", [P, HEADS], F16, kind="ExternalInput")
    # packed per-channel constants replicated across partitions:
    # [bias(64) | gamma(64) | beta(64) | prelu_w(1)]
    crep = nc.dram_tensor("crep", [P, 3 * OUT_DIM + 1], F32, kind="ExternalInput")
    out = nc.dram_tensor("out", [NDP, OUT_DIM], F32, kind="ExternalOutput")

    with tile.TileContext(nc) as tc, ExitStack() as ctx:
        const_p = ctx.enter_context(tc.tile_pool(name="const", bufs=1))
        xet_p = ctx.enter_context(tc.tile_pool(name="xet", bufs=3))
        rhs_p = ctx.enter_context(tc.tile_pool(name="rhs", bufs=2))
        ach_p = ctx.enter_context(tc.tile_pool(name="ach", bufs=2))
        epi_p = ctx.enter_context(tc.tile_pool(name="epi", bufs=1))
        ph_p = ctx.enter_context(tc.tile_pool(name="ph", bufs=2, space="PSUM"))
        pm_p = ctx.enter_context(tc.tile_pool(name="pm", bufs=3, space="PSUM"))
        pa_p = ctx.enter_context(tc.tile_pool(name="pa", bufs=1, space="PSUM"))

        # ---- constants ----
        w_s = const_p.tile([P, HC], F16)
        nc.sync.dma_start(w_s[:], W16d[:])
        v_s = const_p.tile([P, HEADS], F16)
        nc.sync.dma_start(v_s[:], V16d[:])
        u_s = const_p.tile([P, HEADS], F16)
        nc.sync.dma_start(u_s[:], U16d[:])
        cr_s = const_p.tile([P, 3 * OUT_DIM + 1], F32)
        nc.sync.dma_start(cr_s[:], crep[:])
        w_prelu = cr_s[:, 3 * OUT_DIM:3 * OUT_DIM + 1]

        # big accumulators for the batched epilogue
        acc_all = const_p.tile([P, NB, RW], F32)      # raw psum copies

        # ---- main loop ----
        # tile -> (block, is_first_in_block, is_last_in_block)
        tinfo = []
        for b, nt in enumerate(T_b):
            for ti in range(nt):
                tinfo.append((b, ti == 0, ti == nt - 1))

        # ramped chunk sizes: small first chunks so the edge pipeline starts
        # before the full stream depth is resident (start is DMA-contended)
        bounds = [0, 8, 24]
        while bounds[-1] + CH < n_tiles:
            bounds.append(bounds[-1] + CH)
        bounds.append(n_tiles)
        if bounds[-1] == bounds[-2]:
            bounds.pop()
        nchunks_r = len(bounds) - 1

        def load_dma(c):
            lo = bounds[c] * P
            hi = bounds[c + 1] * P
            w = hi - lo
            xet_ch = xet_p.tile([P, CH * P], F16, tag="xet")
            nc.sync.dma_start(xet_ch[:, :w], xeT[:, lo:hi])
            sm_ch = xet_p.tile([P, CH * P], F8, tag="smask")
            nc.sync.dma_start(sm_ch[:, :w], smaskd[:, lo:hi])
            smt_ch = xet_p.tile([P, CH * P], F8, tag="smt")
            nc.sync.dma_start(smt_ch[:, :w], smtd[:, lo:hi])
            return xet_ch, sm_ch, smt_ch

        def process_chunk(c):
            ctiles = bounds[c + 1] - bounds[c]
            xet_ch, sm_ch, smt_ch = dma_cache.pop(c) if c in dma_cache \
                else load_dma(c)
            if c == 0:
                # same ordering trick: chunk-1's rings fire after chunk-0's
                ot = ach_p.tile([P, 1], F8, tag="ord")
                nc.sync.dma_start(ot[:], smt_ch[:, 0:1])
            # prefetch the next chunk's streams (bufs=3 keeps DMA ahead)
            if c + 1 < nchunks_r and c + 1 not in dma_cache:
                dma_cache[c + 1] = load_dma(c + 1)

            # alpha for the whole chunk: one PSUM bank, element-wise groups
            pa = pa_p.tile([P, CH * HEADS], F32, space="PSUM")
            for ti in range(ctiles):
                t = bounds[c] + ti
                b = tinfo[t][0]
                asl = slice(ti * HEADS, (ti + 1) * HEADS)
                nc.tensor.matmul(pa[:, asl], lhsT=xet_ch[:, ti * P:(ti + 1) * P],
                                 rhs=v_s[:],
                                 start=(ti == 0), stop=False,
                                 skip_group_check=True)
                nc.tensor.matmul(
                    pa[:, asl], lhsT=smt_ch[:, ti * P:(ti + 1) * P],
                    rhs=adst_s[:, b * HEADS:(b + 1) * HEADS],
                    start=False, stop=(ti == ctiles - 1),
                    skip_group_check=True)
            # chunk-batched leaky + exp (one ACT instr, no table switches in
            # steady state); STT can read only one PSUM operand, so copy out
            a_ch = ach_p.tile([P, CH * HEADS], F32, tag="a_ch")
            nc.scalar.copy(a_ch[:, :ctiles * HEADS], pa[:, :ctiles * HEADS])
            lk_ch = ach_p.tile([P, CH * HEADS], F32, tag="lk_ch")
            nc.vector.scalar_tensor_tensor(
                out=lk_ch[:, :ctiles * HEADS], in0=a_ch[:, :ctiles * HEADS],
                scalar=NEG_SLOPE, in1=a_ch[:, :ctiles * HEADS],
                op0=mybir.AluOpType.mult, op1=mybir.AluOpType.max)
            e_ch = ach_p.tile([P, CH * HEADS], F32, tag="e_ch")
            nc.scalar.activation(e_ch[:, :ctiles * HEADS],
                                 lk_ch[:, :ctiles * HEADS],
                                 mybir.ActivationFunctionType.Exp)
            # denom columns for every tile of the chunk: one strided ACT copy
            # into the big per-chunk aggregation operand (table-free)
            rhs_ch = rhs_p.tile([P, CH, RW], F16, tag="rhs")
            nc.scalar.copy(
                rhs_ch[:, 0:ctiles, HC:RW],
                e_ch[:, :ctiles * HEADS].rearrange("p (t h) -> p t h", h=HEADS))
            return xet_ch, sm_ch, e_ch, rhs_ch

        # ---- phase 0: a_dst for local nodes (kept in SBUF, fp16) ----
        # (chunk-0 streams start first so their DMA overlaps phase-0 compute)
        adst_s = const_p.tile([P, NB * HEADS], F16)
        with tc.tile_pool(name="p0", bufs=1) as p0_pool:
            xtl_s = p0_pool.tile([P, NDP], F16)
            nc.sync.dma_start(xtl_s[:], xTl[:])
            # ordering shim: this tiny DMA *waits* on xTl's completion, so
            # every chunk dma_start queued behind it on the sync engine fires
            # only after xTl's ring has drained -- the early streams stop
            # contending for HBM bandwidth (phase-0 start was DMA-gated)
            order_t = p0_pool.tile([P, 1], F16)
            nc.sync.dma_start(order_t[:], xtl_s[:, 0:1])
            dma_cache = {}
            dma_cache[0] = load_dma(0)
            for b in range(NB):
                ps = pm_p.tile([P, HEADS], F32, space="PSUM", tag="pm")
                nc.tensor.matmul(
                    ps[:], lhsT=xtl_s[:, b * P:(b + 1) * P], rhs=u_s[:],
                    start=True, stop=True)
                nc.scalar.copy(adst_s[:, b * HEADS:(b + 1) * HEADS], ps[:])

        # per-block-group epilogue bulk (head-mean via softmax reciprocals,
        # square + LN reduces), emitted inside the main loop right after a
        # group's blocks finish so it fills DVE idle windows
        rec = epi_p.tile([P, NB, HEADS], F32)
        macc = epi_p.tile([P, NB, OUT_DIM], F32)
        tmp = epi_p.tile([P, NB, OUT_DIM], F32)
        ssum = epi_p.tile([P, NB], F32)
        ssq = epi_p.tile([P, NB], F32)
        mean = epi_p.tile([P, NB], F32)
        var = epi_p.tile([P, NB], F32)
        m2 = epi_p.tile([P, NB], F32)
        rstd = epi_p.tile([P, NB], F32)
        eps_s = epi_p.tile([P, 1], F32)
        nc.vector.memset(eps_s[:], EPS)

        out_ap_full = bass.AP(out.ap().tensor, 0,
                              [[OUT_DIM, P], [P * OUT_DIM, NB], [1, OUT_DIM]])

        def emit_tail(g0, g1):
            """mean/var -> rstd -> normalize -> PReLU -> store, for blocks
            [g0, g1). One Sqrt per call (one activation-table pair swap)."""
            hb = slice(g0, g1)
            w = g1 - g0
            nc.vector.tensor_scalar(out=mean[:, hb], in0=ssum[:, hb],
                                    scalar1=1.0 / OUT_DIM, scalar2=None,
                                    op0=mybir.AluOpType.mult)
            nc.vector.tensor_scalar(out=var[:, hb], in0=ssq[:, hb],
                                    scalar1=1.0 / OUT_DIM, scalar2=None,
                                    op0=mybir.AluOpType.mult)
            nc.vector.tensor_tensor(out=m2[:, hb], in0=mean[:, hb],
                                    in1=mean[:, hb], op=mybir.AluOpType.mult)
            nc.vector.tensor_tensor(out=var[:, hb], in0=var[:, hb],
                                    in1=m2[:, hb], op=mybir.AluOpType.subtract)
            nc.scalar.activation(rstd[:, hb], var[:, hb],
                                 mybir.ActivationFunctionType.Sqrt,
                                 bias=eps_s[:, 0:1])
            nc.vector.reciprocal(rstd[:, hb], rstd[:, hb])
            mean_b = bass.AP(mean[:].tensor, mean[:].offset + g0,
                             [mean[:].ap[0], [1, w], [0, OUT_DIM]])
            rstd_b = bass.AP(rstd[:].tensor, rstd[:].offset + g0,
                             [rstd[:].ap[0], [1, w], [0, OUT_DIM]])
            nc.vector.tensor_tensor(out=macc[:, hb, :], in0=macc[:, hb, :],
                                    in1=mean_b, op=mybir.AluOpType.subtract)
            nc.vector.tensor_tensor(out=macc[:, hb, :], in0=macc[:, hb, :],
                                    in1=rstd_b, op=mybir.AluOpType.mult)
            if not triv:
                gamma_b = bass.AP(cr_s[:].tensor, cr_s[:].offset + OUT_DIM,
                                  [cr_s[:].ap[0], [0, w], [1, OUT_DIM]])
                beta_b = bass.AP(cr_s[:].tensor, cr_s[:].offset + 2 * OUT_DIM,
                                 [cr_s[:].ap[0], [0, w], [1, OUT_DIM]])
                nc.vector.tensor_tensor(out=macc[:, hb, :], in0=macc[:, hb, :],
                                        in1=gamma_b, op=mybir.AluOpType.mult)
                nc.vector.tensor_tensor(out=macc[:, hb, :], in0=macc[:, hb, :],
                                        in1=beta_b, op=mybir.AluOpType.add)
            # PReLU with 0 < pw < 1: max(y, pw*y)
            nc.vector.scalar_tensor_tensor(
                out=macc[:, hb, :], in0=macc[:, hb, :], scalar=pw,
                in1=macc[:, hb, :], op0=mybir.AluOpType.mult,
                op1=mybir.AluOpType.max)
            out_slice = bass.AP(out_ap_full.tensor, g0 * P * OUT_DIM,
                                [[OUT_DIM, P], [P * OUT_DIM, w], [1, OUT_DIM]])
            nc.sync.dma_start(out_slice, macc[:, hb, :])

        def rec_ap(hd, g0, g1):
            base = rec[:]
            return bass.AP(base.tensor, base.offset + g0 * HEADS + hd,
                           [base.ap[0], [HEADS, g1 - g0], [0, OUT_DIM]])

        def emit_group(g0, g1):
            hb = slice(g0, g1)
            nc.vector.reciprocal(rec[:, hb, :], acc_all[:, hb, HC:RW])
            nc.vector.tensor_tensor(out=macc[:, hb, :],
                                    in0=acc_all[:, hb, 0:OUT_DIM],
                                    in1=rec_ap(0, g0, g1),
                                    op=mybir.AluOpType.mult)
            for hd in range(1, HEADS):
                nc.vector.tensor_tensor(
                    out=tmp[:, hb, :],
                    in0=acc_all[:, hb, hd * OUT_DIM:(hd + 1) * OUT_DIM],
                    in1=rec_ap(hd, g0, g1), op=mybir.AluOpType.mult)
                nc.vector.tensor_add(macc[:, hb, :], macc[:, hb, :],
                                     tmp[:, hb, :])
            if not triv:
                bias_b = bass.AP(cr_s[:].tensor, cr_s[:].offset,
                                 [cr_s[:].ap[0], [0, g1 - g0], [1, OUT_DIM]])
                nc.vector.tensor_tensor(out=macc[:, hb, :], in0=macc[:, hb, :],
                                        in1=bias_b, op=mybir.AluOpType.add)
            nc.vector.tensor_tensor(out=tmp[:, hb, :], in0=macc[:, hb, :],
                                    in1=macc[:, hb, :], op=mybir.AluOpType.mult)
            nc.vector.tensor_reduce(ssum[:, hb], macc[:, hb, :],
                                    mybir.AxisListType.X, mybir.AluOpType.add)
            nc.vector.tensor_reduce(ssq[:, hb], tmp[:, hb, :],
                                    mybir.AxisListType.X, mybir.AluOpType.add)

        GROUPS = (9, 18, 27, 36, 44, 48, NB)
        next_g = 0
        done_g = 0

        processed = {}
        cur_c = -1
        xet_ch = sm_ch = e_ch = rhs_ch = None
        pm = None
        for t0 in range(0, n_tiles, 2):
            npair = min(2, n_tiles - t0)
            if t0 >= bounds[cur_c + 1] if cur_c >= 0 else True:
                c = cur_c + 1 if cur_c >= 0 else 0
                while bounds[c + 1] <= t0:
                    c += 1
                tup = processed.pop(c, None) or process_chunk(c)
                xet_ch, sm_ch, e_ch, rhs_ch = tup
                cur_c = c
            c = cur_c
            toff0 = t0 - bounds[c]
            mid = ((bounds[c] + bounds[c + 1]) // 2) & ~1
            if t0 == mid and c + 1 < nchunks_r and c + 1 not in processed:
                # emit the next chunk's alpha pipeline mid-chunk so none of
                # it sits on the chunk boundary
                processed[c + 1] = process_chunk(c + 1)

            # projections into a bank-aligned PSUM pair tile
            phb = ph_p.tile([P, 2, 512], F32, space="PSUM")
            for j in range(npair):
                sl = slice((toff0 + j) * P, (toff0 + j + 1) * P)
                nc.tensor.matmul(phb[:, j, 0:HC], lhsT=xet_ch[:, sl],
                                 rhs=w_s[:], start=True, stop=True,
                                 skip_group_check=True)

            # rhs[:, :, :256] = h * e (per-head broadcast), one instr per pair
            eb0 = e_ch[:, toff0 * HEADS:(toff0 + npair) * HEADS]
            e_b = bass.AP(eb0.tensor, eb0.offset,
                          [eb0.ap[0], [HEADS, npair], [1, HEADS],
                           [0, OUT_DIM]])
            r0 = rhs_ch[:, toff0:toff0 + npair, 0:HC]
            nc.vector.tensor_tensor(
                out=r0.rearrange("p t (h c) -> p t h c", h=HEADS),
                in0=phb[:, 0:npair, 0:HC].rearrange(
                    "p t (h c) -> p t h c", h=HEADS),
                in1=e_b, op=mybir.AluOpType.mult)

            # segment sums + denominators
            for j in range(npair):
                t = t0 + j
                b, first, last = tinfo[t]
                sl = slice((toff0 + j) * P, (toff0 + j + 1) * P)
                if first:
                    pm = pm_p.tile([P, RW], F32, space="PSUM", tag="pm")
                nc.tensor.matmul(pm[:], lhsT=sm_ch[:, sl],
                                 rhs=rhs_ch[:, toff0 + j, :],
                                 start=first, stop=last)
                if last:
                    nc.scalar.copy(acc_all[:, b, :], pm[:])
                    if b + 1 == GROUPS[next_g]:
                        emit_group(done_g, b + 1)
                        done_g = b + 1
                        next_g += 1
                        if done_g == 44:
                            emit_tail(0, 44)
                        elif done_g == 48:
                            emit_tail(44, 48)

        # ---- epilogue final stage: the last block ----
        emit_tail(48, NB)

    nc.compile()
    return nc


def _prep(x, edge_index, W, att_src, att_dst, bias, gamma, beta, prelu_w):
    """Host-side sharding: self-loops, dst-sort, per-core per-block padding,
    per-edge-slot source-feature expansion (fp16), one-hot mask streams,
    weight folding."""
    src = np.concatenate([edge_index[0], np.arange(N, dtype=edge_index.dtype)])
    dst = np.concatenate([edge_index[1], np.arange(N, dtype=edge_index.dtype)])
    order = np.argsort(dst, kind="stable")
    src = src[order].astype(np.int64)
    dst = dst[order].astype(np.int64)

    # folded attention vectors: a_src = x @ V, a_dst = x @ U
    Wh = W.reshape(IN_DIM, HEADS, OUT_DIM)
    V = np.einsum("khc,hc->kh", Wh, att_src).astype(np.float64)  # [128, H]
    U = np.einsum("khc,hc->kh", Wh, att_dst)                     # [128, H]

    # pad column q: q @ V = -c for every head -> exp weight == 0
    c = 5000.0
    Q, _, _, _ = np.linalg.lstsq(V.T, -c * np.ones(HEADS), rcond=None)
    q16 = Q.astype(np.float16)
    assert np.all(np.isfinite(q16)), "pad vector overflows fp16"
    assert (q16.astype(np.float64) @ V < -500).all(), "pad logits not low enough"

    x16 = x.astype(np.float16)

    # degree-balanced dst placement: assign destinations to (core, block)
    # bins so per-bin edge counts equalize -- the shared tile budget T_b is
    # set by the per-block max across cores, so balance cuts padding tiles.
    import heapq
    deg = np.bincount(dst, minlength=N).astype(np.int64)   # incl. self-loop
    order_d = np.argsort(-deg, kind="stable")
    heap = [(0, k, b) for k in range(NCORES) for b in range(NB)]
    heapq.heapify(heap)
    free = np.full((NCORES, NB), P, dtype=np.int64)
    free[:, NB - 1] = ND - (NB - 1) * P        # last block: 106 real dsts
    core_of_d = np.empty(N, dtype=np.int64)
    blk_of_d = np.empty(N, dtype=np.int64)
    pos_of_d = np.empty(N, dtype=np.int64)
    spill = []
    for d_ in order_d:
        while True:
            s, k, b = heapq.heappop(heap)
            if free[k, b] > 0:
                break
            spill.append((s, k, b))
        core_of_d[d_] = k
        blk_of_d[d_] = b
        pos_of_d[d_] = P - ((ND - (NB - 1) * P) if b == NB - 1 else P) \
            + 0  # placeholder, real pos assigned below
        free[k, b] -= 1
        heapq.heappush(heap, (s + deg[d_], k, b))
    # positions within each bin: stable order of assignment
    pos_of_d[:] = 0
    for k in range(NCORES):
        for b in range(NB):
            sel = np.where((core_of_d == k) & (blk_of_d == b))[0]
            pos_of_d[sel] = np.arange(len(sel))

    core_of = core_of_d[dst]
    counts = np.zeros((NCORES, NB), dtype=np.int64)
    np.add.at(counts, (core_of, blk_of_d[dst]), 1)
    T_b = tuple(int(v) for v in np.ceil(counts.max(axis=0) / P).astype(np.int64))
    S = int(sum(T_b)) * P

    in_maps = []
    W16 = W.astype(np.float16)
    V16 = V.astype(np.float16)
    U16 = U.astype(np.float16)
    crep = np.zeros((P, 3 * OUT_DIM + 1), dtype=np.float32)
    crep[:, 0:OUT_DIM] = bias
    crep[:, OUT_DIM:2 * OUT_DIM] = gamma
    crep[:, 2 * OUT_DIM:3 * OUT_DIM] = beta
    crep[:, 3 * OUT_DIM] = prelu_w[0]

    slot_starts = np.concatenate([[0], np.cumsum(np.array(T_b) * P)])
    import ml_dtypes
    eye8 = np.eye(P, dtype=ml_dtypes.float8_e4m3)
    for k in range(NCORES):
        sel = core_of == k
        src_k, dst_k = src[sel], dst[sel]
        blk_k = blk_of_d[dst_k]

        src_slots = np.zeros(S, dtype=np.int64)
        pad_mask = np.ones(S, dtype=bool)
        dloc = np.full(S, 127, dtype=np.int64)
        o = np.argsort(blk_k, kind="stable")
        src_k, dst_k, blk_k = src_k[o], dst_k[o], blk_k[o]
        bstart = np.searchsorted(blk_k, np.arange(NB + 1))
        for b in range(NB):
            lo, hi = bstart[b], bstart[b + 1]
            n = hi - lo
            s0 = slot_starts[b]
            src_slots[s0:s0 + n] = src_k[lo:hi]
            pad_mask[s0:s0 + n] = False
            dloc[s0:s0 + n] = pos_of_d[dst_k[lo:hi]]

        xe = x16[src_slots]                          # [S, 128]
        xe[pad_mask] = q16
        xeT = np.ascontiguousarray(xe.T)             # [128, S]

        # one-hot masks, both orientations, tile-major along free dim
        oh = eye8[dloc].reshape(S // P, P, P)       # [t, e, d]
        smask = np.ascontiguousarray(
            oh.transpose(1, 0, 2).reshape(P, S))     # [e, (t d)]
        smt = np.ascontiguousarray(
            oh.transpose(2, 0, 1).reshape(P, S))     # [d, (t e)]

        xTl = np.zeros((P, NDP), dtype=np.float16)
        mine = np.where(core_of_d == k)[0]
        rows = blk_of_d[mine] * P + pos_of_d[mine]
        xTl[:, rows] = x16[mine].T

        in_maps.append({
            "xeT": xeT, "smask": smask, "smt": smt, "xTl": xTl,
            "W16": W16, "V16": V16, "U16": U16, "crep": crep,
        })
    outidx = core_of_d * NDP + blk_of_d * P + pos_of_d
    return S, T_b, in_maps, outidx


def kernel(x, edge_index, W, att_src, att_dst, bias, gamma, beta, prelu_w,
           _trace=False):
    x = np.asarray(x, dtype=np.float32)
    edge_index = np.asarray(edge_index)
    S, T_b, in_maps, outidx = _prep(
        x, edge_index, np.asarray(W, np.float32), np.asarray(att_src, np.float32),
        np.asarray(att_dst, np.float32), np.asarray(bias, np.float32),
        np.asarray(gamma, np.float32), np.asarray(beta, np.float32),
        np.asarray(prelu_w, np.float32))

    pw = float(np.asarray(prelu_w).reshape(-1)[0])
    triv = bool(np.all(np.asarray(bias) == 0) and np.all(np.asarray(gamma) == 1)
                and np.all(np.asarray(beta) == 0))
    assert 0.0 < pw < 1.0, "max-form PReLU requires 0 < w < 1"
    key = (S, T_b, pw, triv)
    if key not in _CACHE:
        _CACHE[key] = _build(S, T_b, pw, triv)
    nc = _CACHE[key]

    res = run_bass_kernel_spmd(nc, in_maps, core_ids=list(range(NCORES)),
                               trace=_trace)
    allout = np.concatenate(
        [res.results[k]["out"] for k in range(NCORES)], axis=0)
    out = allout[outidx]
    if _trace:
        kernel.last_exec_time_ns = res.exec_time_ns
    return out


# revision 34
# speedup vs baseline: 1.0094x; 1.0073x over previous
"""GAT layer (project + edge-softmax attention + aggregate + head-mean + LayerNorm + PReLU)
on 8 Trainium2 NeuronCores.

Sharding: nodes/edges partitioned by destination across the 8 cores; edges of
each core are grouped into 128-destination blocks and 128-edge tiles, tiles
into 32-tile streamed chunks. Per chunk the attention logits are batched:
alpha = x_e@V + smt.T@a_dst accumulates in one PSUM bank via two small PE
matmuls per tile, then one chunk-level leaky-relu (DVE STT) + exp (scalar
engine, single activation table -> no table reloads) produce e; a strided
scalar-engine copy drops the softmax-denominator columns of every tile into
the per-chunk aggregation operand. The whole alpha pipeline for chunk c+1 is
emitted at the midpoint of chunk c so none of it sits on a chunk boundary.
Per tile the projection h_e = x[src_e] @ W runs on PE into bank-aligned PSUM
pair tiles [128, 2, 512]; one DVE broadcast multiply per PAIR forms the
weighted messages h_e * e for both tiles (amortizing the ~120-cycle PSUM read
overhead), and a one-hot mask matmul accumulates messages + denominators per
destination block. The epilogue is interleaved: per block-group (reciprocal,
head-mean, LayerNorm reduces) on DVE as soon as a group's blocks finish, with
a two-stage tail (normalize + PReLU + output DMA for blocks 0-44 hidden under
the last tiles; remainder after the loop). LayerNorm scale-invariance absorbs
the 1/HEADS head-mean factor; trivial affine constants (bias=0, gamma=1,
beta=0) and the PReLU weight are baked at compile time (cache-keyed), with
PReLU as max(y, w*y) for 0 < w < 1.

The host side (input sharding) expands source features per edge slot
(x.T[:, src[slot]], fp16) and ships the one-hot destination masks as fp8
(exact 0/1 index data) so the device consumes purely sequential streams --
per-edge DMA gathers are descriptor-rate-bound (~14 ns/descriptor measured)
on TRN2 and cannot reach the memory roofline, and on-device mask construction
is DVE-bound.
"""
import sys

sys.path.insert(0, "/opt/trn_rl_repo")

import numpy as np
from contextlib import ExitStack

import concourse.bass as bass
import concourse.tile as tile
from concourse import bacc, mybir
from concourse.bass_utils import run_bass_kernel_spmd

# ---- problem constants (hardcoded per harness contract) ----
N = 50000
IN_DIM = 128
OUT_DIM = 64
HEADS = 4
HC = HEADS * OUT_DIM          # 256
NEG_SLOPE = 0.2
EPS = 1e-5

NCORES = 8
ND = N // NCORES              # 6250 dst nodes per core
P = 128
NB = (ND + P - 1) // P        # 49 blocks (last has 106 dsts)
NDP = NB * P                  # 6272 padded local nodes
CH = 32                       # tiles per streamed chunk

F16 = mybir.dt.float16
F32 = mybir.dt.float32
F8 = mybir.dt.float8e4

_CACHE = {}


def _build(S, T_b, pw, triv):
    """Compile the SPMD program. S = padded edge slots per core (mult of 128),
    T_b = tuple of per-block tile counts (len NB, sum*128 == S), pw = PReLU
    weight baked as an immediate (0 < pw < 1 required by the max-form),
    triv = bias==0 & gamma==1 & beta==0 (skips the corresponding epilogue
    ops)."""
    n_tiles = S // P
    RW = HC + HEADS           # 260: rhs/psum width (256 msg + 4 denom cols)

    nc = bacc.Bacc("TRN2", target_bir_lowering=False, debug=False)

    xeT = nc.dram_tensor("xeT", [P, S], F16, kind="ExternalInput")
    smaskd = nc.dram_tensor("smask", [P, S], F8, kind="ExternalInput")
    smtd = nc.dram_tensor("smt", [P, S], F8, kind="ExternalInput")
    xTl = nc.dram_tensor("xTl", [P, NDP], F16, kind="ExternalInput")
    W16d = nc.dram_tensor("W16", [P, HC], F16, kind="ExternalInput")
    V16d = nc.dram_tensor("V16", [P, HEADS], F16, kind="ExternalInput")
    U16d = nc.dram_tensor("U16", [P, HEADS], F16, kind="ExternalInput")
    # packed per-channel constants replicated across partitions:
    # [bias(64) | gamma(64) | beta(64) | prelu_w(1)]
    crep = nc.dram_tensor("crep", [P, 3 * OUT_DIM + 1], F32, kind="ExternalInput")
    out = nc.dram_tensor("out", [NDP, OUT_DIM], F32, kind="ExternalOutput")

    with tile.TileContext(nc) as tc, ExitStack() as ctx:
        const_p = ctx.enter_context(tc.tile_pool(name="const", bufs=1))
        xet_p = ctx.enter_context(tc.tile_pool(name="xet", bufs=3))
        rhs_p = ctx.enter_context(tc.tile_pool(name="rhs", bufs=2))
        ach_p = ctx.enter_context(tc.tile_pool(name="ach", bufs=2))
        epi_p = ctx.enter_context(tc.tile_pool(name="epi", bufs=1))
        ph_p = ctx.enter_context(tc.tile_pool(name="ph", bufs=2, space="PSUM"))
        pm_p = ctx.enter_context(tc.tile_pool(name="pm", bufs=2, space="PSUM"))
        pa_p = ctx.enter_context(tc.tile_pool(name="pa", bufs=2, space="PSUM"))

        # ---- constants ----
        w_s = const_p.tile([P, HC], F16)
        nc.sync.dma_start(w_s[:], W16d[:])
        v_s = const_p.tile([P, HEADS], F16)
        nc.sync.dma_start(v_s[:], V16d[:])
        u_s = const_p.tile([P, HEADS], F16)
        nc.sync.dma_start(u_s[:], U16d[:])
        cr_s = const_p.tile([P, 3 * OUT_DIM + 1], F32)
        nc.sync.dma_start(cr_s[:], crep[:])
        w_prelu = cr_s[:, 3 * OUT_DIM:3 * OUT_DIM + 1]

        # big accumulators for the batched epilogue
        acc_all = const_p.tile([P, NB, RW], F32)      # raw psum copies

        # ---- main loop ----
        # tile -> (block, is_first_in_block, is_last_in_block)
        tinfo = []
        for b, nt in enumerate(T_b):
            for ti in range(nt):
                tinfo.append((b, ti == 0, ti == nt - 1))

        # ramped chunk sizes: small first chunks so the edge pipeline starts
        # before the full stream depth is resident (start is DMA-contended)
        bounds = [0, 8, 24]
        while bounds[-1] + CH < n_tiles:
            bounds.append(bounds[-1] + CH)
        bounds.append(n_tiles)
        if bounds[-1] == bounds[-2]:
            bounds.pop()
        nchunks_r = len(bounds) - 1

        def load_dma(c):
            lo = bounds[c] * P
            hi = bounds[c + 1] * P
            w = hi - lo
            xet_ch = xet_p.tile([P, CH * P], F16, tag="xet")
            nc.sync.dma_start(xet_ch[:, :w], xeT[:, lo:hi])
            sm_ch = xet_p.tile([P, CH * P], F8, tag="smask")
            nc.sync.dma_start(sm_ch[:, :w], smaskd[:, lo:hi])
            smt_ch = xet_p.tile([P, CH * P], F8, tag="smt")
            nc.sync.dma_start(smt_ch[:, :w], smtd[:, lo:hi])
            return xet_ch, sm_ch, smt_ch

        def process_chunk(c):
            ctiles = bounds[c + 1] - bounds[c]
            xet_ch, sm_ch, smt_ch = dma_cache.pop(c) if c in dma_cache \
                else load_dma(c)
            if c == 0:
                # same ordering trick: chunk-1's rings fire after chunk-0's
                ot = ach_p.tile([P, 1], F8, tag="ord")
                nc.sync.dma_start(ot[:], smt_ch[:, 0:1])
            # prefetch the next chunk's streams (bufs=3 keeps DMA ahead)
            if c + 1 < nchunks_r and c + 1 not in dma_cache:
                dma_cache[c + 1] = load_dma(c + 1)

            # alpha for the whole chunk: one PSUM bank, element-wise groups
            pa = pa_p.tile([P, CH * HEADS], F32, space="PSUM")
            for ti in range(ctiles):
                t = bounds[c] + ti
                b = tinfo[t][0]
                asl = slice(ti * HEADS, (ti + 1) * HEADS)
                nc.tensor.matmul(pa[:, asl], lhsT=xet_ch[:, ti * P:(ti + 1) * P],
                                 rhs=v_s[:],
                                 start=(ti == 0), stop=False,
                                 skip_group_check=True)
                nc.tensor.matmul(
                    pa[:, asl], lhsT=smt_ch[:, ti * P:(ti + 1) * P],
                    rhs=adst_s[:, b * HEADS:(b + 1) * HEADS],
                    start=False, stop=(ti == ctiles - 1),
                    skip_group_check=True)
            # chunk-batched leaky + exp (one ACT instr, no table switches in
            # steady state); STT can read only one PSUM operand, so copy out
            a_ch = ach_p.tile([P, CH * HEADS], F32, tag="a_ch")
            nc.scalar.copy(a_ch[:, :ctiles * HEADS], pa[:, :ctiles * HEADS])
            lk_ch = ach_p.tile([P, CH * HEADS], F32, tag="lk_ch")
            nc.vector.scalar_tensor_tensor(
                out=lk_ch[:, :ctiles * HEADS], in0=a_ch[:, :ctiles * HEADS],
                scalar=NEG_SLOPE, in1=a_ch[:, :ctiles * HEADS],
                op0=mybir.AluOpType.mult, op1=mybir.AluOpType.max)
            e_ch = ach_p.tile([P, CH * HEADS], F32, tag="e_ch")
            nc.scalar.activation(e_ch[:, :ctiles * HEADS],
                                 lk_ch[:, :ctiles * HEADS],
                                 mybir.ActivationFunctionType.Exp)
            # denom columns for every tile of the chunk: one strided ACT copy
            # into the big per-chunk aggregation operand (table-free)
            rhs_ch = rhs_p.tile([P, CH, RW], F16, tag="rhs")
            nc.scalar.copy(
                rhs_ch[:, 0:ctiles, HC:RW],
                e_ch[:, :ctiles * HEADS].rearrange("p (t h) -> p t h", h=HEADS))
            return xet_ch, sm_ch, e_ch, rhs_ch

        # ---- phase 0: a_dst for local nodes (kept in SBUF, fp16) ----
        # (chunk-0 streams start first so their DMA overlaps phase-0 compute)
        adst_s = const_p.tile([P, NB * HEADS], F16)
        with tc.tile_pool(name="p0", bufs=1) as p0_pool:
            xtl_s = p0_pool.tile([P, NDP], F16)
            nc.sync.dma_start(xtl_s[:], xTl[:])
            # ordering shim: this tiny DMA *waits* on xTl's completion, so
            # every chunk dma_start queued behind it on the sync engine fires
            # only after xTl's ring has drained -- the early streams stop
            # contending for HBM bandwidth (phase-0 start was DMA-gated)
            order_t = p0_pool.tile([P, 1], F16)
            nc.sync.dma_start(order_t[:], xtl_s[:, 0:1])
            dma_cache = {}
            dma_cache[0] = load_dma(0)
            for b in range(NB):
                ps = pm_p.tile([P, HEADS], F32, space="PSUM", tag="pm")
                nc.tensor.matmul(
                    ps[:], lhsT=xtl_s[:, b * P:(b + 1) * P], rhs=u_s[:],
                    start=True, stop=True)
                nc.scalar.copy(adst_s[:, b * HEADS:(b + 1) * HEADS], ps[:])

        # per-block-group epilogue bulk (head-mean via softmax reciprocals,
        # square + LN reduces), emitted inside the main loop right after a
        # group's blocks finish so it fills DVE idle windows
        rec = epi_p.tile([P, NB, HEADS], F32)
        macc = epi_p.tile([P, NB, OUT_DIM], F32)
        tmp = epi_p.tile([P, NB, OUT_DIM], F32)
        ssum = epi_p.tile([P, NB], F32)
        ssq = epi_p.tile([P, NB], F32)
        mean = epi_p.tile([P, NB], F32)
        var = epi_p.tile([P, NB], F32)
        m2 = epi_p.tile([P, NB], F32)
        rstd = epi_p.tile([P, NB], F32)
        eps_s = epi_p.tile([P, 1], F32)
        nc.vector.memset(eps_s[:], EPS)

        out_ap_full = bass.AP(out.ap().tensor, 0,
                              [[OUT_DIM, P], [P * OUT_DIM, NB], [1, OUT_DIM]])

        def emit_tail(g0, g1):
            """mean/var -> rstd -> normalize -> PReLU -> store, for blocks
            [g0, g1). One Sqrt per call (one activation-table pair swap)."""
            hb = slice(g0, g1)
            w = g1 - g0
            nc.vector.tensor_scalar(out=mean[:, hb], in0=ssum[:, hb],
                                    scalar1=1.0 / OUT_DIM, scalar2=None,
                                    op0=mybir.AluOpType.mult)
            nc.vector.tensor_scalar(out=var[:, hb], in0=ssq[:, hb],
                                    scalar1=1.0 / OUT_DIM, scalar2=None,
                                    op0=mybir.AluOpType.mult)
            nc.vector.tensor_tensor(out=m2[:, hb], in0=mean[:, hb],
                                    in1=mean[:, hb], op=mybir.AluOpType.mult)
            nc.vector.tensor_tensor(out=var[:, hb], in0=var[:, hb],
                                    in1=m2[:, hb], op=mybir.AluOpType.subtract)
            nc.scalar.activation(rstd[:, hb], var[:, hb],
                                 mybir.ActivationFunctionType.Sqrt,
                                 bias=eps_s[:, 0:1])
            nc.vector.reciprocal(rstd[:, hb], rstd[:, hb])
            mean_b = bass.AP(mean[:].tensor, mean[:].offset + g0,
                             [mean[:].ap[0], [1, w], [0, OUT_DIM]])
            rstd_b = bass.AP(rstd[:].tensor, rstd[:].offset + g0,
                             [rstd[:].ap[0], [1, w], [0, OUT_DIM]])
            nc.vector.tensor_tensor(out=macc[:, hb, :], in0=macc[:, hb, :],
                                    in1=mean_b, op=mybir.AluOpType.subtract)
            nc.vector.tensor_tensor(out=macc[:, hb, :], in0=macc[:, hb, :],
                                    in1=rstd_b, op=mybir.AluOpType.mult)
            if not triv:
                gamma_b = bass.AP(cr_s[:].tensor, cr_s[:].offset + OUT_DIM,
                                  [cr_s[:].ap[0], [0, w], [1, OUT_DIM]])
                beta_b = bass.AP(cr_s[:].tensor, cr_s[:].offset + 2 * OUT_DIM,
                                 [cr_s[:].ap[0], [0, w], [1, OUT_DIM]])
                nc.vector.tensor_tensor(out=macc[:, hb, :], in0=macc[:, hb, :],
                                        in1=gamma_b, op=mybir.AluOpType.mult)
                nc.vector.tensor_tensor(out=macc[:, hb, :], in0=macc[:, hb, :],
                                        in1=beta_b, op=mybir.AluOpType.add)
            # PReLU with 0 < pw < 1: max(y, pw*y)
            nc.vector.scalar_tensor_tensor(
                out=macc[:, hb, :], in0=macc[:, hb, :], scalar=pw,
                in1=macc[:, hb, :], op0=mybir.AluOpType.mult,
                op1=mybir.AluOpType.max)
            out_slice = bass.AP(out_ap_full.tensor, g0 * P * OUT_DIM,
                                [[OUT_DIM, P], [P * OUT_DIM, w], [1, OUT_DIM]])
            nc.sync.dma_start(out_slice, macc[:, hb, :])

        def rec_ap(hd, g0, g1):
            base = rec[:]
            return bass.AP(base.tensor, base.offset + g0 * HEADS + hd,
                           [base.ap[0], [HEADS, g1 - g0], [0, OUT_DIM]])

        def emit_group(g0, g1):
            hb = slice(g0, g1)
            nc.vector.reciprocal(rec[:, hb, :], acc_all[:, hb, HC:RW])
            nc.vector.tensor_tensor(out=macc[:, hb, :],
                                    in0=acc_all[:, hb, 0:OUT_DIM],
                                    in1=rec_ap(0, g0, g1),
                                    op=mybir.AluOpType.mult)
            for hd in range(1, HEADS):
                nc.vector.tensor_tensor(
                    out=tmp[:, hb, :],
                    in0=acc_all[:, hb, hd * OUT_DIM:(hd + 1) * OUT_DIM],
                    in1=rec_ap(hd, g0, g1), op=mybir.AluOpType.mult)
                nc.vector.tensor_add(macc[:, hb, :], macc[:, hb, :],
                                     tmp[:, hb, :])
            if not triv:
                bias_b = bass.AP(cr_s[:].tensor, cr_s[:].offset,
                                 [cr_s[:].ap[0], [0, g1 - g0], [1, OUT_DIM]])
                nc.vector.tensor_tensor(out=macc[:, hb, :], in0=macc[:, hb, :],
                                        in1=bias_b, op=mybir.AluOpType.add)
            nc.vector.tensor_tensor(out=tmp[:, hb, :], in0=macc[:, hb, :],
                                    in1=macc[:, hb, :], op=mybir.AluOpType.mult)
            nc.vector.tensor_reduce(ssum[:, hb], macc[:, hb, :],
                                    mybir.AxisListType.X, mybir.AluOpType.add)
            nc.vector.tensor_reduce(ssq[:, hb], tmp[:, hb, :],
                                    mybir.AxisListType.X, mybir.AluOpType.add)

        GROUPS = (9, 18, 27, 36, 44, 48, NB)
        next_g = 0
        done_g = 0

        processed = {}
        cur_c = -1
        xet_ch = sm_ch = e_ch = rhs_ch = None
        pm = None
        for t0 in range(0, n_tiles, 2):
            npair = min(2, n_tiles - t0)
            if t0 >= bounds[cur_c + 1] if cur_c >= 0 else True:
                c = cur_c + 1 if cur_c >= 0 else 0
                while bounds[c + 1] <= t0:
                    c += 1
                tup = processed.pop(c, None) or process_chunk(c)
                xet_ch, sm_ch, e_ch, rhs_ch = tup
                cur_c = c
            c = cur_c
            toff0 = t0 - bounds[c]
            mid = ((bounds[c] + bounds[c + 1]) // 2) & ~1
            if t0 == mid and c + 1 < nchunks_r and c + 1 not in processed:
                # emit the next chunk's alpha pipeline mid-chunk so none of
                # it sits on the chunk boundary
                processed[c + 1] = process_chunk(c + 1)

            # projections into a bank-aligned PSUM pair tile
            phb = ph_p.tile([P, 2, 512], F32, space="PSUM")
            for j in range(npair):
                sl = slice((toff0 + j) * P, (toff0 + j + 1) * P)
                nc.tensor.matmul(phb[:, j, 0:HC], lhsT=xet_ch[:, sl],
                                 rhs=w_s[:], start=True, stop=True,
                                 skip_group_check=True)

            # rhs[:, :, :256] = h * e (per-head broadcast), one instr per pair
            eb0 = e_ch[:, toff0 * HEADS:(toff0 + npair) * HEADS]
            e_b = bass.AP(eb0.tensor, eb0.offset,
                          [eb0.ap[0], [HEADS, npair], [1, HEADS],
                           [0, OUT_DIM]])
            r0 = rhs_ch[:, toff0:toff0 + npair, 0:HC]
            nc.vector.tensor_tensor(
                out=r0.rearrange("p t (h c) -> p t h c", h=HEADS),
                in0=phb[:, 0:npair, 0:HC].rearrange(
                    "p t (h c) -> p t h c", h=HEADS),
                in1=e_b, op=mybir.AluOpType.mult)

            # segment sums + denominators
            for j in range(npair):
                t = t0 + j
                b, first, last = tinfo[t]
                sl = slice((toff0 + j) * P, (toff0 + j + 1) * P)
                if first:
                    pm = pm_p.tile([P, RW], F32, space="PSUM", tag="pm")
                nc.tensor.matmul(pm[:], lhsT=sm_ch[:, sl],
                                 rhs=rhs_ch[:, toff0 + j, :],
                                 start=first, stop=last)
                if last:
                    nc.scalar.copy(acc_all[:, b, :], pm[:])
                    if b + 1 == GROUPS[next_g]:
                        emit_group(done_g, b + 1)
                        done_g = b + 1
                        next_g += 1
                        if done_g == 44:
                            emit_tail(0, 44)
                        elif done_g == 48:
                            emit_tail(44, 48)

        # ---- epilogue final stage: the last block ----
        emit_tail(48, NB)

    nc.compile()
    return nc


def _prep(x, edge_index, W, att_src, att_dst, bias, gamma, beta, prelu_w):
    """Host-side sharding: self-loops, dst-sort, per-core per-block padding,
    per-edge-slot source-feature expansion (fp16), one-hot mask streams,
    weight folding."""
    src = np.concatenate([edge_index[0], np.arange(N, dtype=edge_index.dtype)])
    dst = np.concatenate([edge_index[1], np.arange(N, dtype=edge_index.dtype)])
    order = np.argsort(dst, kind="stable")
    src = src[order].astype(np.int64)
    dst = dst[order].astype(np.int64)

    # folded attention vectors: a_src = x @ V, a_dst = x @ U
    Wh = W.reshape(IN_DIM, HEADS, OUT_DIM)
    V = np.einsum("khc,hc->kh", Wh, att_src).astype(np.float64)  # [128, H]
    U = np.einsum("khc,hc->kh", Wh, att_dst)                     # [128, H]

    # pad column q: q @ V = -c for every head -> exp weight == 0
    c = 5000.0
    Q, _, _, _ = np.linalg.lstsq(V.T, -c * np.ones(HEADS), rcond=None)
    q16 = Q.astype(np.float16)
    assert np.all(np.isfinite(q16)), "pad vector overflows fp16"
    assert (q16.astype(np.float64) @ V < -500).all(), "pad logits not low enough"

    x16 = x.astype(np.float16)

    # degree-balanced dst placement: assign destinations to (core, block)
    # bins so per-bin edge counts equalize -- the shared tile budget T_b is
    # set by the per-block max across cores, so balance cuts padding tiles.
    import heapq
    deg = np.bincount(dst, minlength=N).astype(np.int64)   # incl. self-loop
    order_d = np.argsort(-deg, kind="stable")
    heap = [(0, k, b) for k in range(NCORES) for b in range(NB)]
    heapq.heapify(heap)
    free = np.full((NCORES, NB), P, dtype=np.int64)
    free[:, NB - 1] = ND - (NB - 1) * P        # last block: 106 real dsts
    core_of_d = np.empty(N, dtype=np.int64)
    blk_of_d = np.empty(N, dtype=np.int64)
    pos_of_d = np.empty(N, dtype=np.int64)
    spill = []
    for d_ in order_d:
        while True:
            s, k, b = heapq.heappop(heap)
            if free[k, b] > 0:
                break
            spill.append((s, k, b))
        core_of_d[d_] = k
        blk_of_d[d_] = b
        pos_of_d[d_] = P - ((ND - (NB - 1) * P) if b == NB - 1 else P) \
            + 0  # placeholder, real pos assigned below
        free[k, b] -= 1
        heapq.heappush(heap, (s + deg[d_], k, b))
    # positions within each bin: stable order of assignment
    pos_of_d[:] = 0
    for k in range(NCORES):
        for b in range(NB):
            sel = np.where((core_of_d == k) & (blk_of_d == b))[0]
            pos_of_d[sel] = np.arange(len(sel))

    core_of = core_of_d[dst]
    counts = np.zeros((NCORES, NB), dtype=np.int64)
    np.add.at(counts, (core_of, blk_of_d[dst]), 1)
    T_b = tuple(int(v) for v in np.ceil(counts.max(axis=0) / P).astype(np.int64))
    S = int(sum(T_b)) * P

    in_maps = []
    W16 = W.astype(np.float16)
    V16 = V.astype(np.float16)
    U16 = U.astype(np.float16)
    crep = np.zeros((P, 3 * OUT_DIM + 1), dtype=np.float32)
    crep[:, 0:OUT_DIM] = bias
    crep[:, OUT_DIM:2 * OUT_DIM] = gamma
    crep[:, 2 * OUT_DIM:3 * OUT_DIM] = beta
    crep[:, 3 * OUT_DIM] = prelu_w[0]

    slot_starts = np.concatenate([[0], np.cumsum(np.array(T_b) * P)])
    import ml_dtypes
    eye8 = np.eye(P, dtype=ml_dtypes.float8_e4m3)
    for k in range(NCORES):
        sel = core_of == k
        src_k, dst_k = src[sel], dst[sel]
        blk_k = blk_of_d[dst_k]

        src_slots = np.zeros(S, dtype=np.int64)
        pad_mask = np.ones(S, dtype=bool)
        dloc = np.full(S, 127, dtype=np.int64)
        o = np.argsort(blk_k, kind="stable")
        src_k, dst_k, blk_k = src_k[o], dst_k[o], blk_k[o]
        bstart = np.searchsorted(blk_k, np.arange(NB + 1))
        for b in range(NB):
            lo, hi = bstart[b], bstart[b + 1]
            n = hi - lo
            s0 = slot_starts[b]
            src_slots[s0:s0 + n] = src_k[lo:hi]
            pad_mask[s0:s0 + n] = False
            dloc[s0:s0 + n] = pos_of_d[dst_k[lo:hi]]

        xe = x16[src_slots]                          # [S, 128]
        xe[pad_mask] = q16
        xeT = np.ascontiguousarray(xe.T)             # [128, S]

        # one-hot masks, both orientations, tile-major along free dim
        oh = eye8[dloc].reshape(S // P, P, P)       # [t, e, d]
        smask = np.ascontiguousarray(
            oh.transpose(1, 0, 2).reshape(P, S))     # [e, (t d)]
        smt = np.ascontiguousarray(
            oh.transpose(2, 0, 1).reshape(P, S))     # [d, (t e)]

        xTl = np.zeros((P, NDP), dtype=np.float16)
        mine = np.where(core_of_d == k)[0]
        rows = blk_of_d[mine] * P + pos_of_d[mine]
        xTl[:, rows] = x16[mine].T

        in_maps.append({
            "xeT": xeT, "smask": smask, "smt": smt, "xTl": xTl,
            "W16": W16, "V16": V16, "U16": U16, "crep": crep,
        })
    outidx = core_of_d * NDP + blk_of_d * P + pos_of_d
    return S, T_b, in_maps, outidx


def kernel(x, edge_index, W, att_src, att_dst, bias, gamma, beta, prelu_w,
           _trace=False):
    x = np.asarray(x, dtype=np.float32)
    edge_index = np.asarray(edge_index)
    S, T_b, in_maps, outidx = _prep(
        x, edge_index, np.asarray(W, np.float32), np.asarray(att_src, np.float32),
        np.asarray(att_dst, np.float32), np.asarray(bias, np.float32),
        np.asarray(gamma, np.float32), np.asarray(beta, np.float32),
        np.asarray(prelu_w, np.float32))

    pw = float(np.asarray(prelu_w).reshape(-1)[0])
    triv = bool(np.all(np.asarray(bias) == 0) and np.all(np.asarray(gamma) == 1)
                and np.all(np.asarray(beta) == 0))
    assert 0.0 < pw < 1.0, "max-form PReLU requires 0 < w < 1"
    key = (S, T_b, pw, triv)
    if key not in _CACHE:
        _CACHE[key] = _build(S, T_b, pw, triv)
    nc = _CACHE[key]

    res = run_bass_kernel_spmd(nc, in_maps, core_ids=list(range(NCORES)),
                               trace=_trace)
    allout = np.concatenate(
        [res.results[k]["out"] for k in range(NCORES)], axis=0)
    out = allout[outidx]
    if _trace:
        kernel.last_exec_time_ns = res.exec_time_ns
    return out


# revision 35
# speedup vs baseline: 1.0253x; 1.0158x over previous
"""GAT layer (project + edge-softmax attention + aggregate + head-mean + LayerNorm + PReLU)
on 8 Trainium2 NeuronCores.

Sharding: nodes/edges partitioned by destination across the 8 cores; edges of
each core are grouped into 128-destination blocks and 128-edge tiles, tiles
into 32-tile streamed chunks. Per chunk the attention logits are batched:
alpha = x_e@V + smt.T@a_dst accumulates in one PSUM bank via two small PE
matmuls per tile, then one chunk-level leaky-relu (DVE STT) + exp (scalar
engine, single activation table -> no table reloads) produce e; a strided
scalar-engine copy drops the softmax-denominator columns of every tile into
the per-chunk aggregation operand. The whole alpha pipeline for chunk c+1 is
emitted at the midpoint of chunk c so none of it sits on a chunk boundary.
Per tile the projection h_e = x[src_e] @ W runs on PE into bank-aligned PSUM
pair tiles [128, 2, 512]; one DVE broadcast multiply per PAIR forms the
weighted messages h_e * e for both tiles (amortizing the ~120-cycle PSUM read
overhead), and a one-hot mask matmul accumulates messages + denominators per
destination block. The epilogue is interleaved: per block-group (reciprocal,
head-mean, LayerNorm reduces) on DVE as soon as a group's blocks finish, with
a two-stage tail (normalize + PReLU + output DMA for blocks 0-44 hidden under
the last tiles; remainder after the loop). LayerNorm scale-invariance absorbs
the 1/HEADS head-mean factor; trivial affine constants (bias=0, gamma=1,
beta=0) and the PReLU weight are baked at compile time (cache-keyed), with
PReLU as max(y, w*y) for 0 < w < 1.

The host side (input sharding) expands source features per edge slot
(x.T[:, src[slot]], fp16) and ships the one-hot destination masks as fp8
(exact 0/1 index data) so the device consumes purely sequential streams --
per-edge DMA gathers are descriptor-rate-bound (~14 ns/descriptor measured)
on TRN2 and cannot reach the memory roofline, and on-device mask construction
is DVE-bound.
"""
import sys

sys.path.insert(0, "/opt/trn_rl_repo")

import numpy as np
from contextlib import ExitStack

import concourse.bass as bass
import concourse.tile as tile
from concourse import bacc, mybir
from concourse.bass_utils import run_bass_kernel_spmd

# ---- problem constants (hardcoded per harness contract) ----
N = 50000
IN_DIM = 128
OUT_DIM = 64
HEADS = 4
HC = HEADS * OUT_DIM          # 256
NEG_SLOPE = 0.2
EPS = 1e-5

NCORES = 8
ND = N // NCORES              # 6250 dst nodes per core
P = 128
NB = (ND + P - 1) // P        # 49 blocks (last has 106 dsts)
NDP = NB * P                  # 6272 padded local nodes
CH = 32                       # tiles per streamed chunk

F16 = mybir.dt.float16
F32 = mybir.dt.float32
F8 = mybir.dt.float8e4

_CACHE = {}


def _build(S, T_b, pw, triv):
    """Compile the SPMD program. S = padded edge slots per core (mult of 128),
    T_b = tuple of per-block tile counts (len NB, sum*128 == S), pw = PReLU
    weight baked as an immediate (0 < pw < 1 required by the max-form),
    triv = bias==0 & gamma==1 & beta==0 (skips the corresponding epilogue
    ops)."""
    n_tiles = S // P
    RW = HC + HEADS           # 260: rhs/psum width (256 msg + 4 denom cols)

    nc = bacc.Bacc("TRN2", target_bir_lowering=False, debug=False)

    xeT = nc.dram_tensor("xeT", [P, S], F16, kind="ExternalInput")
    smaskd = nc.dram_tensor("smask", [P, S], F8, kind="ExternalInput")
    smtd = nc.dram_tensor("smt", [P, S], F8, kind="ExternalInput")
    xTl = nc.dram_tensor("xTl", [P, NDP], F16, kind="ExternalInput")
    W16d = nc.dram_tensor("W16", [P, HC], F16, kind="ExternalInput")
    V16d = nc.dram_tensor("V16", [P, HEADS], F16, kind="ExternalInput")
    U16d = nc.dram_tensor("U16", [P, HEADS], F16, kind="ExternalInput")
    # packed per-channel constants replicated across partitions:
    # [bias(64) | gamma(64) | beta(64) | prelu_w(1)]
    crep = nc.dram_tensor("crep", [P, 3 * OUT_DIM + 1], F32, kind="ExternalInput")
    out = nc.dram_tensor("out", [NDP, OUT_DIM], F32, kind="ExternalOutput")

    with tile.TileContext(nc) as tc, ExitStack() as ctx:
        const_p = ctx.enter_context(tc.tile_pool(name="const", bufs=1))
        xet_p = ctx.enter_context(tc.tile_pool(name="xet", bufs=4))
        rhs_p = ctx.enter_context(tc.tile_pool(name="rhs", bufs=2))
        ach_p = ctx.enter_context(tc.tile_pool(name="ach", bufs=2))
        epi_p = ctx.enter_context(tc.tile_pool(name="epi", bufs=1))
        ph_p = ctx.enter_context(tc.tile_pool(name="ph", bufs=2, space="PSUM"))
        pm_p = ctx.enter_context(tc.tile_pool(name="pm", bufs=2, space="PSUM"))
        pa_p = ctx.enter_context(tc.tile_pool(name="pa", bufs=2, space="PSUM"))

        # ---- constants ----
        w_s = const_p.tile([P, HC], F16)
        nc.sync.dma_start(w_s[:], W16d[:])
        v_s = const_p.tile([P, HEADS], F16)
        nc.sync.dma_start(v_s[:], V16d[:])
        u_s = const_p.tile([P, HEADS], F16)
        nc.sync.dma_start(u_s[:], U16d[:])
        cr_s = const_p.tile([P, 3 * OUT_DIM + 1], F32)
        nc.sync.dma_start(cr_s[:], crep[:])
        w_prelu = cr_s[:, 3 * OUT_DIM:3 * OUT_DIM + 1]

        # big accumulators for the batched epilogue
        acc_all = const_p.tile([P, NB, RW], F32)      # raw psum copies

        # ---- main loop ----
        # tile -> (block, is_first_in_block, is_last_in_block)
        tinfo = []
        for b, nt in enumerate(T_b):
            for ti in range(nt):
                tinfo.append((b, ti == 0, ti == nt - 1))

        # ramped chunk sizes: small first chunks so the edge pipeline starts
        # before the full stream depth is resident (start is DMA-contended)
        bounds = [0, 8, 24, 48]
        while bounds[-1] + CH < n_tiles:
            bounds.append(bounds[-1] + CH)
        bounds.append(n_tiles)
        if bounds[-1] == bounds[-2]:
            bounds.pop()
        nchunks_r = len(bounds) - 1

        def load_dma(c):
            lo = bounds[c] * P
            hi = bounds[c + 1] * P
            w = hi - lo
            xet_ch = xet_p.tile([P, CH * P], F16, tag="xet")
            nc.sync.dma_start(xet_ch[:, :w], xeT[:, lo:hi])
            sm_ch = xet_p.tile([P, CH * P], F8, tag="smask")
            nc.sync.dma_start(sm_ch[:, :w], smaskd[:, lo:hi])
            smt_ch = xet_p.tile([P, CH * P], F8, tag="smt")
            nc.sync.dma_start(smt_ch[:, :w], smtd[:, lo:hi])
            return xet_ch, sm_ch, smt_ch

        def process_chunk(c):
            ctiles = bounds[c + 1] - bounds[c]
            xet_ch, sm_ch, smt_ch = dma_cache.pop(c) if c in dma_cache \
                else load_dma(c)
            if c == 0:
                # same ordering trick: chunk-1's rings fire after chunk-0's
                ot = ach_p.tile([P, 1], F8, tag="ord")
                nc.sync.dma_start(ot[:], smt_ch[:, 0:1])
            # prefetch the next chunk's streams (bufs=3 keeps DMA ahead)
            if c + 1 < nchunks_r and c + 1 not in dma_cache:
                dma_cache[c + 1] = load_dma(c + 1)

            # alpha for the whole chunk: one PSUM bank, element-wise groups
            pa = pa_p.tile([P, CH * HEADS], F32, space="PSUM")
            for ti in range(ctiles):
                t = bounds[c] + ti
                b = tinfo[t][0]
                asl = slice(ti * HEADS, (ti + 1) * HEADS)
                nc.tensor.matmul(pa[:, asl], lhsT=xet_ch[:, ti * P:(ti + 1) * P],
                                 rhs=v_s[:],
                                 start=(ti == 0), stop=False,
                                 skip_group_check=True)
                nc.tensor.matmul(
                    pa[:, asl], lhsT=smt_ch[:, ti * P:(ti + 1) * P],
                    rhs=adst_s[:, b * HEADS:(b + 1) * HEADS],
                    start=False, stop=(ti == ctiles - 1),
                    skip_group_check=True)
            # chunk-batched leaky + exp (one ACT instr, no table switches in
            # steady state); STT can read only one PSUM operand, so copy out
            a_ch = ach_p.tile([P, CH * HEADS], F32, tag="a_ch")
            nc.scalar.copy(a_ch[:, :ctiles * HEADS], pa[:, :ctiles * HEADS])
            lk_ch = ach_p.tile([P, CH * HEADS], F32, tag="lk_ch")
            nc.vector.scalar_tensor_tensor(
                out=lk_ch[:, :ctiles * HEADS], in0=a_ch[:, :ctiles * HEADS],
                scalar=NEG_SLOPE, in1=a_ch[:, :ctiles * HEADS],
                op0=mybir.AluOpType.mult, op1=mybir.AluOpType.max)
            e_ch = ach_p.tile([P, CH * HEADS], F32, tag="e_ch")
            nc.scalar.activation(e_ch[:, :ctiles * HEADS],
                                 lk_ch[:, :ctiles * HEADS],
                                 mybir.ActivationFunctionType.Exp)
            # denom columns for every tile of the chunk: one strided ACT copy
            # into the big per-chunk aggregation operand (table-free)
            rhs_ch = rhs_p.tile([P, CH, RW], F16, tag="rhs")
            nc.scalar.copy(
                rhs_ch[:, 0:ctiles, HC:RW],
                e_ch[:, :ctiles * HEADS].rearrange("p (t h) -> p t h", h=HEADS))
            return xet_ch, sm_ch, e_ch, rhs_ch

        # ---- phase 0: a_dst for local nodes (kept in SBUF, fp16) ----
        # (chunk-0 streams start first so their DMA overlaps phase-0 compute)
        adst_s = const_p.tile([P, NB * HEADS], F16)
        with tc.tile_pool(name="p0", bufs=1) as p0_pool:
            xtl_s = p0_pool.tile([P, NDP], F16)
            nc.sync.dma_start(xtl_s[:], xTl[:])
            # ordering shim: this tiny DMA *waits* on xTl's completion, so
            # every chunk dma_start queued behind it on the sync engine fires
            # only after xTl's ring has drained -- the early streams stop
            # contending for HBM bandwidth (phase-0 start was DMA-gated)
            order_t = p0_pool.tile([P, 1], F16)
            nc.sync.dma_start(order_t[:], xtl_s[:, 0:1])
            dma_cache = {}
            dma_cache[0] = load_dma(0)
            for b in range(NB):
                ps = pm_p.tile([P, HEADS], F32, space="PSUM", tag="pm")
                nc.tensor.matmul(
                    ps[:], lhsT=xtl_s[:, b * P:(b + 1) * P], rhs=u_s[:],
                    start=True, stop=True)
                nc.scalar.copy(adst_s[:, b * HEADS:(b + 1) * HEADS], ps[:])

        # per-block-group epilogue bulk (head-mean via softmax reciprocals,
        # square + LN reduces), emitted inside the main loop right after a
        # group's blocks finish so it fills DVE idle windows
        rec = epi_p.tile([P, NB, HEADS], F32)
        macc = epi_p.tile([P, NB, OUT_DIM], F32)
        tmp = epi_p.tile([P, NB, OUT_DIM], F32)
        ssum = epi_p.tile([P, NB], F32)
        ssq = epi_p.tile([P, NB], F32)
        mean = epi_p.tile([P, NB], F32)
        var = epi_p.tile([P, NB], F32)
        m2 = epi_p.tile([P, NB], F32)
        rstd = epi_p.tile([P, NB], F32)
        eps_s = epi_p.tile([P, 1], F32)
        nc.vector.memset(eps_s[:], EPS)

        out_ap_full = bass.AP(out.ap().tensor, 0,
                              [[OUT_DIM, P], [P * OUT_DIM, NB], [1, OUT_DIM]])

        def emit_tail(g0, g1):
            """mean/var -> rstd -> normalize -> PReLU -> store, for blocks
            [g0, g1). One Sqrt per call (one activation-table pair swap)."""
            hb = slice(g0, g1)
            w = g1 - g0
            nc.vector.tensor_scalar(out=mean[:, hb], in0=ssum[:, hb],
                                    scalar1=1.0 / OUT_DIM, scalar2=None,
                                    op0=mybir.AluOpType.mult)
            nc.vector.tensor_scalar(out=var[:, hb], in0=ssq[:, hb],
                                    scalar1=1.0 / OUT_DIM, scalar2=None,
                                    op0=mybir.AluOpType.mult)
            nc.vector.tensor_tensor(out=m2[:, hb], in0=mean[:, hb],
                                    in1=mean[:, hb], op=mybir.AluOpType.mult)
            nc.vector.tensor_tensor(out=var[:, hb], in0=var[:, hb],
                                    in1=m2[:, hb], op=mybir.AluOpType.subtract)
            nc.scalar.activation(rstd[:, hb], var[:, hb],
                                 mybir.ActivationFunctionType.Sqrt,
                                 bias=eps_s[:, 0:1])
            nc.vector.reciprocal(rstd[:, hb], rstd[:, hb])
            mean_b = bass.AP(mean[:].tensor, mean[:].offset + g0,
                             [mean[:].ap[0], [1, w], [0, OUT_DIM]])
            rstd_b = bass.AP(rstd[:].tensor, rstd[:].offset + g0,
                             [rstd[:].ap[0], [1, w], [0, OUT_DIM]])
            nc.vector.tensor_tensor(out=macc[:, hb, :], in0=macc[:, hb, :],
                                    in1=mean_b, op=mybir.AluOpType.subtract)
            nc.vector.tensor_tensor(out=macc[:, hb, :], in0=macc[:, hb, :],
                                    in1=rstd_b, op=mybir.AluOpType.mult)
            if not triv:
                gamma_b = bass.AP(cr_s[:].tensor, cr_s[:].offset + OUT_DIM,
                                  [cr_s[:].ap[0], [0, w], [1, OUT_DIM]])
                beta_b = bass.AP(cr_s[:].tensor, cr_s[:].offset + 2 * OUT_DIM,
                                 [cr_s[:].ap[0], [0, w], [1, OUT_DIM]])
                nc.vector.tensor_tensor(out=macc[:, hb, :], in0=macc[:, hb, :],
                                        in1=gamma_b, op=mybir.AluOpType.mult)
                nc.vector.tensor_tensor(out=macc[:, hb, :], in0=macc[:, hb, :],
                                        in1=beta_b, op=mybir.AluOpType.add)
            # PReLU with 0 < pw < 1: max(y, pw*y)
            nc.vector.scalar_tensor_tensor(
                out=macc[:, hb, :], in0=macc[:, hb, :], scalar=pw,
                in1=macc[:, hb, :], op0=mybir.AluOpType.mult,
                op1=mybir.AluOpType.max)
            out_slice = bass.AP(out_ap_full.tensor, g0 * P * OUT_DIM,
                                [[OUT_DIM, P], [P * OUT_DIM, w], [1, OUT_DIM]])
            nc.sync.dma_start(out_slice, macc[:, hb, :])

        def rec_ap(hd, g0, g1):
            base = rec[:]
            return bass.AP(base.tensor, base.offset + g0 * HEADS + hd,
                           [base.ap[0], [HEADS, g1 - g0], [0, OUT_DIM]])

        def emit_group(g0, g1):
            hb = slice(g0, g1)
            nc.vector.reciprocal(rec[:, hb, :], acc_all[:, hb, HC:RW])
            nc.vector.tensor_tensor(out=macc[:, hb, :],
                                    in0=acc_all[:, hb, 0:OUT_DIM],
                                    in1=rec_ap(0, g0, g1),
                                    op=mybir.AluOpType.mult)
            for hd in range(1, HEADS):
                nc.vector.tensor_tensor(
                    out=tmp[:, hb, :],
                    in0=acc_all[:, hb, hd * OUT_DIM:(hd + 1) * OUT_DIM],
                    in1=rec_ap(hd, g0, g1), op=mybir.AluOpType.mult)
                nc.vector.tensor_add(macc[:, hb, :], macc[:, hb, :],
                                     tmp[:, hb, :])
            if not triv:
                bias_b = bass.AP(cr_s[:].tensor, cr_s[:].offset,
                                 [cr_s[:].ap[0], [0, g1 - g0], [1, OUT_DIM]])
                nc.vector.tensor_tensor(out=macc[:, hb, :], in0=macc[:, hb, :],
                                        in1=bias_b, op=mybir.AluOpType.add)
            nc.vector.tensor_tensor(out=tmp[:, hb, :], in0=macc[:, hb, :],
                                    in1=macc[:, hb, :], op=mybir.AluOpType.mult)
            nc.vector.tensor_reduce(ssum[:, hb], macc[:, hb, :],
                                    mybir.AxisListType.X, mybir.AluOpType.add)
            nc.vector.tensor_reduce(ssq[:, hb], tmp[:, hb, :],
                                    mybir.AxisListType.X, mybir.AluOpType.add)

        GROUPS = (9, 18, 27, 36, 44, 48, NB)
        next_g = 0
        done_g = 0

        processed = {}
        cur_c = -1
        xet_ch = sm_ch = e_ch = rhs_ch = None
        pm = None
        for t0 in range(0, n_tiles, 2):
            npair = min(2, n_tiles - t0)
            if t0 >= bounds[cur_c + 1] if cur_c >= 0 else True:
                c = cur_c + 1 if cur_c >= 0 else 0
                while bounds[c + 1] <= t0:
                    c += 1
                tup = processed.pop(c, None) or process_chunk(c)
                xet_ch, sm_ch, e_ch, rhs_ch = tup
                cur_c = c
            c = cur_c
            toff0 = t0 - bounds[c]
            mid = ((bounds[c] + bounds[c + 1]) // 2) & ~1
            if t0 == mid and c + 1 < nchunks_r and c + 1 not in processed:
                # emit the next chunk's alpha pipeline mid-chunk so none of
                # it sits on the chunk boundary
                processed[c + 1] = process_chunk(c + 1)

            # projections into a bank-aligned PSUM pair tile
            phb = ph_p.tile([P, 2, 512], F32, space="PSUM")
            for j in range(npair):
                sl = slice((toff0 + j) * P, (toff0 + j + 1) * P)
                nc.tensor.matmul(phb[:, j, 0:HC], lhsT=xet_ch[:, sl],
                                 rhs=w_s[:], start=True, stop=True,
                                 skip_group_check=True)

            # rhs[:, :, :256] = h * e (per-head broadcast), one instr per pair
            eb0 = e_ch[:, toff0 * HEADS:(toff0 + npair) * HEADS]
            e_b = bass.AP(eb0.tensor, eb0.offset,
                          [eb0.ap[0], [HEADS, npair], [1, HEADS],
                           [0, OUT_DIM]])
            r0 = rhs_ch[:, toff0:toff0 + npair, 0:HC]
            nc.vector.tensor_tensor(
                out=r0.rearrange("p t (h c) -> p t h c", h=HEADS),
                in0=phb[:, 0:npair, 0:HC].rearrange(
                    "p t (h c) -> p t h c", h=HEADS),
                in1=e_b, op=mybir.AluOpType.mult)

            # segment sums + denominators
            for j in range(npair):
                t = t0 + j
                b, first, last = tinfo[t]
                sl = slice((toff0 + j) * P, (toff0 + j + 1) * P)
                if first:
                    pm = pm_p.tile([P, RW], F32, space="PSUM", tag="pm")
                nc.tensor.matmul(pm[:], lhsT=sm_ch[:, sl],
                                 rhs=rhs_ch[:, toff0 + j, :],
                                 start=first, stop=last)
                if last:
                    nc.scalar.copy(acc_all[:, b, :], pm[:])
                    if b + 1 == GROUPS[next_g]:
                        emit_group(done_g, b + 1)
                        done_g = b + 1
                        next_g += 1
                        if done_g == 44:
                            emit_tail(0, 44)
                        elif done_g == 48:
                            emit_tail(44, 48)

        # ---- epilogue final stage: the last block ----
        emit_tail(48, NB)

    nc.compile()
    return nc


def _prep(x, edge_index, W, att_src, att_dst, bias, gamma, beta, prelu_w):
    """Host-side sharding: self-loops, dst-sort, per-core per-block padding,
    per-edge-slot source-feature expansion (fp16), one-hot mask streams,
    weight folding."""
    src = np.concatenate([edge_index[0], np.arange(N, dtype=edge_index.dtype)])
    dst = np.concatenate([edge_index[1], np.arange(N, dtype=edge_index.dtype)])
    order = np.argsort(dst, kind="stable")
    src = src[order].astype(np.int64)
    dst = dst[order].astype(np.int64)

    # folded attention vectors: a_src = x @ V, a_dst = x @ U
    Wh = W.reshape(IN_DIM, HEADS, OUT_DIM)
    V = np.einsum("khc,hc->kh", Wh, att_src).astype(np.float64)  # [128, H]
    U = np.einsum("khc,hc->kh", Wh, att_dst)                     # [128, H]

    # pad column q: q @ V = -c for every head -> exp weight == 0
    c = 5000.0
    Q, _, _, _ = np.linalg.lstsq(V.T, -c * np.ones(HEADS), rcond=None)
    q16 = Q.astype(np.float16)
    assert np.all(np.isfinite(q16)), "pad vector overflows fp16"
    assert (q16.astype(np.float64) @ V < -500).all(), "pad logits not low enough"

    x16 = x.astype(np.float16)

    # degree-balanced dst placement: assign destinations to (core, block)
    # bins so per-bin edge counts equalize -- the shared tile budget T_b is
    # set by the per-block max across cores, so balance cuts padding tiles.
    import heapq
    deg = np.bincount(dst, minlength=N).astype(np.int64)   # incl. self-loop
    order_d = np.argsort(-deg, kind="stable")
    heap = [(0, k, b) for k in range(NCORES) for b in range(NB)]
    heapq.heapify(heap)
    free = np.full((NCORES, NB), P, dtype=np.int64)
    free[:, NB - 1] = ND - (NB - 1) * P        # last block: 106 real dsts
    core_of_d = np.empty(N, dtype=np.int64)
    blk_of_d = np.empty(N, dtype=np.int64)
    pos_of_d = np.empty(N, dtype=np.int64)
    spill = []
    for d_ in order_d:
        while True:
            s, k, b = heapq.heappop(heap)
            if free[k, b] > 0:
                break
            spill.append((s, k, b))
        core_of_d[d_] = k
        blk_of_d[d_] = b
        pos_of_d[d_] = P - ((ND - (NB - 1) * P) if b == NB - 1 else P) \
            + 0  # placeholder, real pos assigned below
        free[k, b] -= 1
        heapq.heappush(heap, (s + deg[d_], k, b))
    # positions within each bin: stable order of assignment
    pos_of_d[:] = 0
    for k in range(NCORES):
        for b in range(NB):
            sel = np.where((core_of_d == k) & (blk_of_d == b))[0]
            pos_of_d[sel] = np.arange(len(sel))

    core_of = core_of_d[dst]
    counts = np.zeros((NCORES, NB), dtype=np.int64)
    np.add.at(counts, (core_of, blk_of_d[dst]), 1)
    T_b = tuple(int(v) for v in np.ceil(counts.max(axis=0) / P).astype(np.int64))
    S = int(sum(T_b)) * P

    in_maps = []
    W16 = W.astype(np.float16)
    V16 = V.astype(np.float16)
    U16 = U.astype(np.float16)
    crep = np.zeros((P, 3 * OUT_DIM + 1), dtype=np.float32)
    crep[:, 0:OUT_DIM] = bias
    crep[:, OUT_DIM:2 * OUT_DIM] = gamma
    crep[:, 2 * OUT_DIM:3 * OUT_DIM] = beta
    crep[:, 3 * OUT_DIM] = prelu_w[0]

    slot_starts = np.concatenate([[0], np.cumsum(np.array(T_b) * P)])
    import ml_dtypes
    eye8 = np.eye(P, dtype=ml_dtypes.float8_e4m3)
    for k in range(NCORES):
        sel = core_of == k
        src_k, dst_k = src[sel], dst[sel]
        blk_k = blk_of_d[dst_k]

        src_slots = np.zeros(S, dtype=np.int64)
        pad_mask = np.ones(S, dtype=bool)
        dloc = np.full(S, 127, dtype=np.int64)
        o = np.argsort(blk_k, kind="stable")
        src_k, dst_k, blk_k = src_k[o], dst_k[o], blk_k[o]
        bstart = np.searchsorted(blk_k, np.arange(NB + 1))
        for b in range(NB):
            lo, hi = bstart[b], bstart[b + 1]
            n = hi - lo
            s0 = slot_starts[b]
            src_slots[s0:s0 + n] = src_k[lo:hi]
            pad_mask[s0:s0 + n] = False
            dloc[s0:s0 + n] = pos_of_d[dst_k[lo:hi]]

        xe = x16[src_slots]                          # [S, 128]
        xe[pad_mask] = q16
        xeT = np.ascontiguousarray(xe.T)             # [128, S]

        # one-hot masks, both orientations, tile-major along free dim
        oh = eye8[dloc].reshape(S // P, P, P)       # [t, e, d]
        smask = np.ascontiguousarray(
            oh.transpose(1, 0, 2).reshape(P, S))     # [e, (t d)]
        smt = np.ascontiguousarray(
            oh.transpose(2, 0, 1).reshape(P, S))     # [d, (t e)]

        xTl = np.zeros((P, NDP), dtype=np.float16)
        mine = np.where(core_of_d == k)[0]
        rows = blk_of_d[mine] * P + pos_of_d[mine]
        xTl[:, rows] = x16[mine].T

        in_maps.append({
            "xeT": xeT, "smask": smask, "smt": smt, "xTl": xTl,
            "W16": W16, "V16": V16, "U16": U16, "crep": crep,
        })
    outidx = core_of_d * NDP + blk_of_d * P + pos_of_d
    return S, T_b, in_maps, outidx


def kernel(x, edge_index, W, att_src, att_dst, bias, gamma, beta, prelu_w,
           _trace=False):
    x = np.asarray(x, dtype=np.float32)
    edge_index = np.asarray(edge_index)
    S, T_b, in_maps, outidx = _prep(
        x, edge_index, np.asarray(W, np.float32), np.asarray(att_src, np.float32),
        np.asarray(att_dst, np.float32), np.asarray(bias, np.float32),
        np.asarray(gamma, np.float32), np.asarray(beta, np.float32),
        np.asarray(prelu_w, np.float32))

    pw = float(np.asarray(prelu_w).reshape(-1)[0])
    triv = bool(np.all(np.asarray(bias) == 0) and np.all(np.asarray(gamma) == 1)
                and np.all(np.asarray(beta) == 0))
    assert 0.0 < pw < 1.0, "max-form PReLU requires 0 < w < 1"
    key = (S, T_b, pw, triv)
    if key not in _CACHE:
        _CACHE[key] = _build(S, T_b, pw, triv)
    nc = _CACHE[key]

    res = run_bass_kernel_spmd(nc, in_maps, core_ids=list(range(NCORES)),
                               trace=_trace)
    allout = np.concatenate(
        [res.results[k]["out"] for k in range(NCORES)], axis=0)
    out = allout[outidx]
    if _trace:
        kernel.last_exec_time_ns = res.exec_time_ns
    return out
